# revision 7
# baseline (speedup 1.0000x reference)
"""Trainium2 Bass kernel for nn_BestHits: out = bh * bh.T where
bh = blockwise-softmax(mask_diag(similarities) / TAU) over 256-wide column groups.

Strategy: out is symmetric (out.T = bh.T * bh = out), so only the upper
triangle of 512x512 block-pairs is computed on device. The 16x16 block grid
has 136 upper-incl-diagonal pairs = 17 per core on 8 cores (each core gets
exactly 2 diagonal + 15 off-diagonal pairs -> perfectly uniform SPMD work).
B-side blocks are staged pre-transposed by the host (layout-only, free).

v3 (measured-rate driven; v1 was 142.6us with ACT 113.6/DVE 111/DMA 104.7
walls; v2's tensor_scalar+accum experiment measured: TT/TS at 2x with
all-16-bit packed operands, accum-TS stuck at 1x + READ_ACCUMULATOR,
GpSimd TT at ~2.1ns/elem):

  * Inputs staged fp16 on the host (free): 16 MiB loads/core vs 32.
  * One merged [P, side, t-pair, B] exp per t-pair covers BOTH the A and
    BT halves in a single big ACTIVATE (2 per off slot, 1137ns/1024e rate)
    with bias=-30 folded in: exp(x/TAU - 30) rescales both softmax
    numerator and denominator consistently (out invariant) and keeps
    W = za*zbt below bf16 overflow for unclamped N(0,1) inputs.
  * A-side group sums as a 2-level bf16 pairwise tree (two 2x
    tensor_tensor adds) + one 1x tensor_reduce over the last 64: ~1.5us
    vs 2.2us flat reduce, vs 3.9us accum-TS, vs 5.4us ACT-accum.
  * Product out = (za*zbt) * (ra x rp): W = za*zbt (2x TT), scale tile
    S[p,t,c] = ra[p,t,g(c)]*rp_{g(t)}[c] built on GpSimd as 8 small
    tensor_scalar_muls (f32 rp row * f32 ra per-partition scalar -> bf16),
    final = W*S (2x TT). DVE does 2 passes at 0.52ns/elem instead of
    8 1x scalar_tensor_tensors.
  * Diagonal slots: ra-apply (wa) moved to ACT (Copy activation with
    per-partition scale), PE transpose path unchanged.
  * Stores on the GpSimd DMA ring, deferred one slot.

Per-slot engine budget (off): ACT 4.0us, DVE ~5.5us, GpSimd ~4.9us,
PE ~2.5us, DMA ~3.7us -> projected walls DVE ~91us, others below.

Per-core HBM traffic: 15*1 MiB + 2*0.5 MiB loads + 17*0.5 MiB stores
= 24.5 MiB.
"""
import sys

import numpy as np

sys.path.insert(0, "/opt/trn_rl_repo")

from contextlib import ExitStack

import concourse.bass as bass  # noqa: F401  (registers AP machinery)
import concourse.tile as tile
from concourse import bacc, masks, mybir
from concourse.bass_utils import run_bass_kernel_spmd

N = 8192          # full matrix side
B = 512           # block side
NB = N // B       # 16 blocks per side
P = 128           # SBUF partitions
T = B // P        # 4 row-subtiles per block
GRP = 256         # softmax group width
NG = B // GRP     # 2 groups per block side
TAU = 0.1
NDIAG = 2         # diagonal pairs per core (the last NDIAG slots)
NSLOTS = 17       # block-pairs per core
NOFF = NSLOTS - NDIAG
NCORES = 8
MASK = -60000.0   # pre-masked diagonal value (fp16-representable; exp->0)
EXP_BIAS = -30.0  # exp(x/TAU + EXP_BIAS): overflow headroom for za*zbt

F32 = mybir.dt.float32
F16 = mybir.dt.float16
BF16 = mybir.dt.bfloat16

AF = mybir.ActivationFunctionType
OP = mybir.AluOpType


def core_pairs() -> list[list[tuple[int, int]]]:
    """136 upper-triangle block pairs distributed 17-per-core; the 2 diagonal
    pairs of each core come last (the kernel treats those slots specially)."""
    diag = [(i, i) for i in range(NB)]
    off = [(i, j) for i in range(NB) for j in range(i + 1, NB)]
    cps: list[list[tuple[int, int]]] = [[] for _ in range(NCORES)]
    for idx, p in enumerate(off):
        cps[idx % NCORES].append(p)
    for idx, p in enumerate(diag):
        cps[idx % NCORES].append(p)
    return cps


CORE_PAIRS = core_pairs()


def build():
    """Build + compile the (single-program, 8-core SPMD) Bass kernel."""
    nc = bacc.Bacc(
        "TRN2",
        target_bir_lowering=False,
        debug=False,
        enable_asserts=True,
        num_devices=NCORES,
    )
    ab = nc.dram_tensor("ab", [NOFF, P, 2, T, B], F16, kind="ExternalInput").ap()
    ad = nc.dram_tensor("ad", [NDIAG, P, T, B], F16, kind="ExternalInput").ap()
    o = nc.dram_tensor("o", [NSLOTS, P, T, B], F16, kind="ExternalOutput").ap()

    with tile.TileContext(nc) as tc, ExitStack() as ctx:
        const_pool = ctx.enter_context(tc.tile_pool(name="const", bufs=1))
        ident = const_pool.tile([P, P], BF16)
        masks.make_identity(nc, ident[:])
        # All-ones stationary: one matmul both colsums zbt's partition groups
        # AND broadcasts the result to all 128 PSUM partitions. bf16 so the
        # matmuls run in one pass (fp32 matmul = 2 passes).
        ones_mat = const_pool.tile([P, P], BF16)
        nc.gpsimd.memset(ones_mat[:], 1.0)
        bias_sb = const_pool.tile([P, 1], F32)
        nc.gpsimd.memset(bias_sb[:], EXP_BIAS)

        ab_pool = ctx.enter_context(tc.tile_pool(name="ab_sb", bufs=5))
        ad_pool = ctx.enter_context(tc.tile_pool(name="ad_sb", bufs=2))
        z_pool = ctx.enter_context(tc.tile_pool(name="zab", bufs=4))
        w_pool = ctx.enter_context(tc.tile_pool(name="w", bufs=4))
        s_pool = ctx.enter_context(tc.tile_pool(name="s", bufs=4))
        h_pool = ctx.enter_context(tc.tile_pool(name="h", bufs=4))
        o_pool = ctx.enter_context(tc.tile_pool(name="o_sb", bufs=4))
        st_pool = ctx.enter_context(tc.tile_pool(name="st", bufs=10))
        rp_pool = ctx.enter_context(tc.tile_pool(name="rp", bufs=4))
        dg_pool = ctx.enter_context(tc.tile_pool(name="dg", bufs=2))
        ps_pool = ctx.enter_context(tc.tile_pool(name="ps", bufs=3, space="PSUM"))

        def tree_sums(za, sa, ra):
            """sa[p, t, g] = sum_c za[p, t, g*256+c]; ra = 1/sa.
            Two 2x-mode bf16 pairwise-add stages + one small 1x reduce."""
            za4 = za.rearrange("p t (g c) -> p (t g) c", c=GRP)
            h1 = h_pool.tile([P, T * NG, GRP // 2], BF16)
            h2 = h_pool.tile([P, T * NG, GRP // 4], BF16)
            nc.vector.tensor_tensor(h1[:], za4[:, :, 0:128], za4[:, :, 128:256],
                                    op=OP.add)
            nc.vector.tensor_tensor(h2[:], h1[:, :, 0:64], h1[:, :, 64:128],
                                    op=OP.add)
            nc.vector.tensor_reduce(sa.rearrange("p t g -> p (t g)"), h2[:],
                                    axis=mybir.AxisListType.X, op=OP.add)
            nc.vector.reciprocal(ra.rearrange("p t g -> p (t g)"),
                                 sa.rearrange("p t g -> p (t g)"))

        # Diagonal slots are interleaved mid-program: their short chains give
        # ACT/DVE low-dependency filler work between full off-slot chains.
        order = [*range(0, 7), NOFF, *range(7, 12), NOFF + 1, *range(12, NOFF)]
        # Stores are deferred one slot: issued immediately, store(k) sits at
        # the GpSimd queue head waiting on slot k's full product and blocks
        # slot k+1's work behind it (head-of-line serialization).
        pending_store = None
        for k in order:
            diag_slot = k >= NOFF
            if not diag_slot:
                # --- off-diagonal pair: A and host-pre-transposed B ---
                ab_sb = ab_pool.tile([P, 2, T, B], F16)
                nc.sync.dma_start(ab_sb[:], ab[k])

                # One merged exp per t-pair covers the A AND BT halves
                # (contiguous in ab_sb); split by t-pair so the PE can start
                # after the first half.
                zab = z_pool.tile([P, 2, T, B], BF16)
                s_ps = ps_pool.tile([P, NG, B], F32, name="p23")
                for h in range(NG):
                    ts = slice(NG * h, NG * (h + 1))
                    nc.scalar.activation(zab[:, :, ts, :], ab_sb[:, :, ts, :],
                                         AF.Exp, scale=1.0 / TAU,
                                         bias=bias_sb[:])
                za = zab[:, 0]
                zbt = zab[:, 1]
                # BT side: ones-matmuls sum each 256-row partition group into
                # PSUM broadcast across all partitions, then rp = 1/sums.
                for g in range(NG):
                    for u in range(NG):
                        nc.tensor.matmul(
                            s_ps[:, g, :], ones_mat[:], zbt[:, g * NG + u, :],
                            start=(u == 0), stop=(u == NG - 1),
                        )
                rp_sb = rp_pool.tile([P, NG, B], F32)
                nc.vector.reciprocal_approx_fast(
                    rp_sb[:].rearrange("p g b -> p (g b)"),
                    s_ps[:].rearrange("p g b -> p (g b)"))

                # A side: group sums + ra.
                sa = st_pool.tile([P, T, NG], F32, name="sa")
                ra = st_pool.tile([P, T, NG], F32, name="ra")
                tree_sums(za, sa[:], ra[:])

                # S[p,t,c] = ra[p,t,g(c)] * rp_{t//2}[c] on GpSimd (8 small
                # tensor_scalar_muls; rp row f32, ra per-partition scalar).
                s_sb = s_pool.tile([P, T, B], BF16)
                for t in range(T):
                    for g in range(NG):
                        cs = slice(g * GRP, (g + 1) * GRP)
                        nc.gpsimd.tensor_scalar_mul(
                            s_sb[:, t, cs], rp_sb[:, t // NG, cs],
                            ra[:, t, g:g + 1])

                # W = za*zbt then out = W*S, both 2x-mode tensor_tensors.
                w_sb = w_pool.tile([P, T, B], BF16)
                o_sb = o_pool.tile([P, T, B], F16)
                nc.vector.tensor_tensor(w_sb[:], za, zbt, op=OP.mult)
                nc.vector.tensor_tensor(o_sb[:], w_sb[:], s_sb[:], op=OP.mult)
            else:
                # --- diagonal pair: B == A, PE bf16 transpose ---
                a_sb = ad_pool.tile([P, T, B], F16)
                nc.sync.dma_start(a_sb[:], ad[k - NOFF])
                zad = z_pool.tile([P, T, B], BF16)
                for h in range(NG):
                    ts = slice(NG * h, NG * (h + 1))
                    nc.scalar.activation(zad[:, ts, :], a_sb[:, ts, :],
                                         AF.Exp, scale=1.0 / TAU,
                                         bias=bias_sb[:])
                sa = st_pool.tile([P, T, NG], F32, name="sa")
                ra = st_pool.tile([P, T, NG], F32, name="ra")
                tree_sums(zad[:], sa[:], ra[:])
                # wa = za*ra on ACT (Copy with per-partition scale); DVE is
                # the bottleneck engine, ACT has headroom.
                wa = w_pool.tile([P, T, B], BF16)
                for t in range(T):
                    for g in range(NG):
                        cs = slice(g * GRP, (g + 1) * GRP)
                        nc.scalar.mul(wa[:, t, cs], zad[:, t, cs],
                                      ra[:, t, g:g + 1])
                dg = dg_pool.tile([P, T * NG, P], BF16)
                nc.gpsimd.tensor_mul(
                    dg[:],
                    ident[:].rearrange("p (one c) -> p one c", one=1)
                    .broadcast_to([P, T * NG, P]),
                    ra[:].rearrange("p t g -> p (t g)")
                    .rearrange("p (tg one) -> p tg one", one=1)
                    .broadcast_to([P, T * NG, P]),
                )
                # Two v-waves through one 2-bank PSUM tile; wave 2 reuses the
                # banks after wave 1's products are read.
                p23 = ps_pool.tile([P, NG, B], F32, name="p23")
                o_sb = o_pool.tile([P, T, B], F16)
                for w in range(NG):
                    for hv in range(NG):
                        v = w * NG + hv
                        for u in range(T):
                            nc.tensor.matmul(
                                p23[:, hv, u * P:(u + 1) * P],
                                zad[:, u, v * P:(v + 1) * P],
                                dg[:, u * NG + (v // NG), :],
                            )
                        nc.vector.tensor_tensor(
                            o_sb[:, v, :], wa[:, v, :], p23[:, hv, :],
                            op=OP.mult)

            # One whole-block store per slot on the SWDGE (gpsimd) ring: it
            # never queues ahead of loads on the sync HWDGE ring.
            if pending_store is not None:
                nc.gpsimd.dma_start(o[pending_store[0]], pending_store[1][:])
            pending_store = (k, o_sb)
        nc.gpsimd.dma_start(o[pending_store[0]], pending_store[1][:])

    nc.compile()
    return nc


_NC = None


def _get_nc():
    global _NC
    if _NC is None:
        _NC = build()
    return _NC


def _to_pmajor(block: np.ndarray) -> np.ndarray:
    # (512, 512) row-major -> (128, 4, 512): row r = t*P + p lands at
    # [p, t, :], so every SBUF partition's bytes are contiguous in DRAM.
    return block.reshape(T, P, B).transpose(1, 0, 2)


def make_in_maps(sims: np.ndarray) -> list[dict[str, np.ndarray]]:
    in_maps = []
    for c in range(NCORES):
        ab_stack = np.empty((NOFF, P, 2, T, B), np.float16)
        ad_stack = np.empty((NDIAG, P, T, B), np.float16)
        for k, (i, j) in enumerate(CORE_PAIRS[c]):
            if k < NOFF:
                assert i != j
                ab_stack[k, :, 0] = _to_pmajor(
                    sims[i * B:(i + 1) * B, j * B:(j + 1) * B]).astype(
                        np.float16)
                ab_stack[k, :, 1] = _to_pmajor(
                    np.ascontiguousarray(
                        sims[j * B:(j + 1) * B, i * B:(i + 1) * B].T)).astype(
                            np.float16)
            else:
                assert i == j
                a = sims[i * B:(i + 1) * B, i * B:(i + 1) * B].copy()
                np.fill_diagonal(a, MASK)
                ad_stack[k - NOFF] = _to_pmajor(a).astype(np.float16)
        in_maps.append({"ab": ab_stack, "ad": ad_stack})
    return in_maps


def assemble(results: list[dict[str, np.ndarray]]) -> np.ndarray:
    out = np.empty((N, N), np.float32)
    for c in range(NCORES):
        o_pm = results[c]["o"]  # (NSLOTS, P, T, B) fp16, partition-major
        o_stack = o_pm.astype(np.float32).transpose(0, 2, 1, 3).reshape(
            NSLOTS, B, B)
        for k, (i, j) in enumerate(CORE_PAIRS[c]):
            out[i * B:(i + 1) * B, j * B:(j + 1) * B] = o_stack[k]
            if i != j:
                out[j * B:(j + 1) * B, i * B:(i + 1) * B] = o_stack[k].T
    return out


def run_on_hw(sims: np.ndarray, **spmd_kwargs):
    """Run the kernel on the 8 NeuronCores. Returns (out, BassKernelResults).

    The device occasionally throws a transient NRT_EXEC_UNIT_UNRECOVERABLE
    and needs ~a minute to come back, so failed runs are retried."""
    import time

    nc = _get_nc()
    in_maps = make_in_maps(sims)
    last_exc = None
    for attempt in range(3):
        if attempt:
            time.sleep(75)
        try:
            res = run_bass_kernel_spmd(
                nc, in_maps, core_ids=list(range(NCORES)), **spmd_kwargs
            )
            return assemble(res.results), res
        except Exception as exc:  # noqa: BLE001 - device flake, retry
            last_exc = exc
    raise last_exc


def kernel(similarities: np.ndarray) -> np.ndarray:
    sims = np.ascontiguousarray(similarities, dtype=np.float32)
    assert sims.shape == (N, N)
    out, _ = run_on_hw(sims)
    return out


if __name__ == "__main__":
    rng = np.random.default_rng(0)
    sims = rng.standard_normal((N, N), dtype=np.float32)
    out = kernel(similarities=sims)
    print("out", out.shape, out.dtype, float(out.max()))


# revision 8
# speedup vs baseline: 3.2390x; 3.2390x over previous
"""Trainium2 Bass kernel for nn_BestHits: out = bh * bh.T where
bh = blockwise-softmax(mask_diag(similarities) / TAU) over 256-wide column groups.

Strategy: out is symmetric (out.T = bh.T * bh = out), so only the upper
triangle of 512x512 block-pairs is computed on device. The 16x16 block grid
has 136 upper-incl-diagonal pairs = 17 per core on 8 cores (each core gets
exactly 2 diagonal + 15 off-diagonal pairs -> perfectly uniform SPMD work).
B-side blocks are staged pre-transposed by the host (layout-only, free).

v3 (measured-rate driven; v1 was 142.6us with ACT 113.6/DVE 111/DMA 104.7
walls; v2's tensor_scalar+accum experiment measured: TT/TS at 2x with
all-16-bit packed operands, accum-TS stuck at 1x + READ_ACCUMULATOR,
GpSimd TT at ~2.1ns/elem):

  * Inputs staged fp16 on the host (free): 16 MiB loads/core vs 32.
  * One merged [P, side, t-pair, B] exp per t-pair covers BOTH the A and
    BT halves in a single big ACTIVATE (2 per off slot, 1137ns/1024e rate)
    with bias=-30 folded in: exp(x/TAU - 30) rescales both softmax
    numerator and denominator consistently (out invariant) and keeps
    W = za*zbt below bf16 overflow for unclamped N(0,1) inputs.
  * A-side group sums as a 2-level bf16 pairwise tree (two 2x
    tensor_tensor adds) + one 1x tensor_reduce over the last 64: ~1.5us
    vs 2.2us flat reduce, vs 3.9us accum-TS, vs 5.4us ACT-accum.
  * Product out = (za*zbt) * (ra x rp): W = za*zbt (2x TT), scale tile
    S[p,t,c] = ra[p,t,g(c)]*rp_{g(t)}[c] built on GpSimd as 8 small
    tensor_scalar_muls (f32 rp row * f32 ra per-partition scalar -> bf16),
    final = W*S (2x TT). DVE does 2 passes at 0.52ns/elem instead of
    8 1x scalar_tensor_tensors.
  * Diagonal slots: ra-apply (wa) moved to ACT (Copy activation with
    per-partition scale), PE transpose path unchanged.
  * Stores on the GpSimd DMA ring, deferred one slot.

Per-slot engine budget (off): ACT 4.0us, DVE ~5.5us, GpSimd ~4.9us,
PE ~2.5us, DMA ~3.7us -> projected walls DVE ~91us, others below.

Per-core HBM traffic: 15*1 MiB + 2*0.5 MiB loads + 17*0.5 MiB stores
= 24.5 MiB.
"""
import sys

import numpy as np

sys.path.insert(0, "/opt/trn_rl_repo")

from contextlib import ExitStack

import concourse.bass as bass  # noqa: F401  (registers AP machinery)
import concourse.tile as tile
from concourse import bacc, masks, mybir
from concourse.bass_utils import run_bass_kernel_spmd

N = 8192          # full matrix side
B = 512           # block side
NB = N // B       # 16 blocks per side
P = 128           # SBUF partitions
T = B // P        # 4 row-subtiles per block
GRP = 256         # softmax group width
NG = B // GRP     # 2 groups per block side
TAU = 0.1
NDIAG = 2         # diagonal pairs per core (the last NDIAG slots)
NSLOTS = 17       # block-pairs per core
NOFF = NSLOTS - NDIAG
NCORES = 8
MASK = -60000.0   # pre-masked diagonal value (fp16-representable; exp->0)
EXP_BIAS = -30.0  # exp(x/TAU + EXP_BIAS): overflow headroom for za*zbt

F32 = mybir.dt.float32
F16 = mybir.dt.float16
BF16 = mybir.dt.bfloat16

AF = mybir.ActivationFunctionType
OP = mybir.AluOpType


def core_pairs() -> list[list[tuple[int, int]]]:
    """136 upper-triangle block pairs distributed 17-per-core; the 2 diagonal
    pairs of each core come last (the kernel treats those slots specially)."""
    diag = [(i, i) for i in range(NB)]
    off = [(i, j) for i in range(NB) for j in range(i + 1, NB)]
    cps: list[list[tuple[int, int]]] = [[] for _ in range(NCORES)]
    for idx, p in enumerate(off):
        cps[idx % NCORES].append(p)
    for idx, p in enumerate(diag):
        cps[idx % NCORES].append(p)
    return cps


CORE_PAIRS = core_pairs()


def build():
    """Build + compile the (single-program, 8-core SPMD) Bass kernel."""
    nc = bacc.Bacc(
        "TRN2",
        target_bir_lowering=False,
        debug=False,
        enable_asserts=True,
        num_devices=NCORES,
    )
    ab = nc.dram_tensor("ab", [NOFF, P, 2, T, B], F16, kind="ExternalInput").ap()
    ad = nc.dram_tensor("ad", [NDIAG, P, T, B], F16, kind="ExternalInput").ap()
    o = nc.dram_tensor("o", [NSLOTS, P, T, B], F16, kind="ExternalOutput").ap()

    with tile.TileContext(nc) as tc, ExitStack() as ctx:
        const_pool = ctx.enter_context(tc.tile_pool(name="const", bufs=1))
        ident = const_pool.tile([P, P], BF16)
        masks.make_identity(nc, ident[:])
        # All-ones stationary: one matmul both colsums zbt's partition groups
        # AND broadcasts the result to all 128 PSUM partitions. bf16 so the
        # matmuls run in one pass (fp32 matmul = 2 passes).
        ones_mat = const_pool.tile([P, P], BF16)
        nc.gpsimd.memset(ones_mat[:], 1.0)
        bias_sb = const_pool.tile([P, 1], F32)
        nc.gpsimd.memset(bias_sb[:], EXP_BIAS)

        ab_pool = ctx.enter_context(tc.tile_pool(name="ab_sb", bufs=5))
        ad_pool = ctx.enter_context(tc.tile_pool(name="ad_sb", bufs=2))
        z_pool = ctx.enter_context(tc.tile_pool(name="zab", bufs=4))
        w_pool = ctx.enter_context(tc.tile_pool(name="w", bufs=4))
        s_pool = ctx.enter_context(tc.tile_pool(name="s", bufs=4))
        h_pool = ctx.enter_context(tc.tile_pool(name="h", bufs=4))
        o_pool = ctx.enter_context(tc.tile_pool(name="o_sb", bufs=4))
        st_pool = ctx.enter_context(tc.tile_pool(name="st", bufs=10))
        rp_pool = ctx.enter_context(tc.tile_pool(name="rp", bufs=4))
        dg_pool = ctx.enter_context(tc.tile_pool(name="dg", bufs=2))
        ps_pool = ctx.enter_context(tc.tile_pool(name="ps", bufs=3, space="PSUM"))

        def tree_sums(za, sa, ra):
            """sa[p, t, g] = sum_c za[p, t, g*256+c]; ra = 1/sa.
            Two 2x-mode bf16 pairwise-add stages + one small 1x reduce."""
            za4 = za.rearrange("p t (g c) -> p (t g) c", c=GRP)
            h1 = h_pool.tile([P, T * NG, GRP // 2], BF16)
            h2 = h_pool.tile([P, T * NG, GRP // 4], BF16)
            nc.vector.tensor_tensor(h1[:], za4[:, :, 0:128], za4[:, :, 128:256],
                                    op=OP.add)
            nc.vector.tensor_tensor(h2[:], h1[:, :, 0:64], h1[:, :, 64:128],
                                    op=OP.add)
            nc.vector.tensor_reduce(sa.rearrange("p t g -> p (t g)"), h2[:],
                                    axis=mybir.AxisListType.X, op=OP.add)
            nc.vector.reciprocal(ra.rearrange("p t g -> p (t g)"),
                                 sa.rearrange("p t g -> p (t g)"))

        # Diagonal slots are interleaved mid-program: their short chains give
        # ACT/DVE low-dependency filler work between full off-slot chains.
        order = [*range(0, 7), NOFF, *range(7, 12), NOFF + 1, *range(12, NOFF)]
        # Stores are deferred one slot: issued immediately, store(k) sits at
        # the GpSimd queue head waiting on slot k's full product and blocks
        # slot k+1's work behind it (head-of-line serialization).
        pending_store = None
        for k in order:
            diag_slot = k >= NOFF
            if not diag_slot:
                # --- off-diagonal pair: A and host-pre-transposed B ---
                ab_sb = ab_pool.tile([P, 2, T, B], F16)
                nc.sync.dma_start(ab_sb[:], ab[k])

                # One merged exp per t-pair covers the A AND BT halves
                # (contiguous in ab_sb); split by t-pair so the PE can start
                # after the first half.
                zab = z_pool.tile([P, 2, T, B], BF16)
                s_ps = ps_pool.tile([P, NG, B], F32, name="p23")
                for h in range(NG):
                    ts = slice(NG * h, NG * (h + 1))
                    nc.scalar.activation(zab[:, :, ts, :], ab_sb[:, :, ts, :],
                                         AF.Exp, scale=1.0 / TAU,
                                         bias=bias_sb[:])
                za = zab[:, 0]
                zbt = zab[:, 1]
                # BT side: ones-matmuls sum each 256-row partition group into
                # PSUM broadcast across all partitions, then rp = 1/sums.
                for g in range(NG):
                    for u in range(NG):
                        nc.tensor.matmul(
                            s_ps[:, g, :], ones_mat[:], zbt[:, g * NG + u, :],
                            start=(u == 0), stop=(u == NG - 1),
                        )
                rp_sb = rp_pool.tile([P, NG, B], F32)
                nc.vector.reciprocal_approx_fast(
                    rp_sb[:].rearrange("p g b -> p (g b)"),
                    s_ps[:].rearrange("p g b -> p (g b)"))

                # A side: group sums + ra.
                sa = st_pool.tile([P, T, NG], F32, name="sa")
                ra = st_pool.tile([P, T, NG], F32, name="ra")
                tree_sums(za, sa[:], ra[:])

                # X = bhB.T in fp16, fully on GpSimd tensor_tensors (2 ops,
                # rp broadcast across the t-pair; GpSimd TT measured
                # 2.25ns/elem -- its tensor_scalar is 7x slower, avoid).
                x_sb = s_pool.tile([P, T, B], F16)
                for h in range(NG):
                    ts = slice(NG * h, NG * (h + 1))
                    nc.gpsimd.tensor_mul(
                        x_sb[:, ts, :], zbt[:, ts, :],
                        rp_sb[:, h:h + 1, :].broadcast_to([P, NG, B]),
                    )

                # wa = za*ra via 2x-mode tensor_scalar_mul (253ns/region
                # measured), then ONE 2x-mode tensor_tensor for out = wa*X.
                wa = w_pool.tile([P, T, B], BF16)
                o_sb = o_pool.tile([P, T, B], F16)
                for t in range(T):
                    for g in range(NG):
                        cs = slice(g * GRP, (g + 1) * GRP)
                        nc.vector.tensor_scalar_mul(
                            wa[:, t, cs], za[:, t, cs], ra[:, t, g:g + 1])
                nc.vector.tensor_tensor(o_sb[:], wa[:], x_sb[:], op=OP.mult)
            else:
                # --- diagonal pair: B == A, PE bf16 transpose ---
                a_sb = ad_pool.tile([P, T, B], F16)
                nc.sync.dma_start(a_sb[:], ad[k - NOFF])
                zad = z_pool.tile([P, T, B], BF16)
                for h in range(NG):
                    ts = slice(NG * h, NG * (h + 1))
                    nc.scalar.activation(zad[:, ts, :], a_sb[:, ts, :],
                                         AF.Exp, scale=1.0 / TAU,
                                         bias=bias_sb[:])
                sa = st_pool.tile([P, T, NG], F32, name="sa")
                ra = st_pool.tile([P, T, NG], F32, name="ra")
                tree_sums(zad[:], sa[:], ra[:])
                wa = w_pool.tile([P, T, B], BF16)
                for t in range(T):
                    for g in range(NG):
                        cs = slice(g * GRP, (g + 1) * GRP)
                        nc.vector.tensor_scalar_mul(
                            wa[:, t, cs], zad[:, t, cs], ra[:, t, g:g + 1])
                dg = dg_pool.tile([P, T * NG, P], BF16)
                nc.gpsimd.tensor_mul(
                    dg[:],
                    ident[:].rearrange("p (one c) -> p one c", one=1)
                    .broadcast_to([P, T * NG, P]),
                    ra[:].rearrange("p t g -> p (t g)")
                    .rearrange("p (tg one) -> p tg one", one=1)
                    .broadcast_to([P, T * NG, P]),
                )
                # Two v-waves through one 2-bank PSUM tile; wave 2 reuses the
                # banks after wave 1's products are read.
                p23 = ps_pool.tile([P, NG, B], F32, name="p23")
                o_sb = o_pool.tile([P, T, B], F16)
                for w in range(NG):
                    for hv in range(NG):
                        v = w * NG + hv
                        for u in range(T):
                            nc.tensor.matmul(
                                p23[:, hv, u * P:(u + 1) * P],
                                zad[:, u, v * P:(v + 1) * P],
                                dg[:, u * NG + (v // NG), :],
                            )
                        nc.vector.tensor_tensor(
                            o_sb[:, v, :], wa[:, v, :], p23[:, hv, :],
                            op=OP.mult)

            # One whole-block store per slot on the SWDGE (gpsimd) ring: it
            # never queues ahead of loads on the sync HWDGE ring.
            if pending_store is not None:
                nc.gpsimd.dma_start(o[pending_store[0]], pending_store[1][:])
            pending_store = (k, o_sb)
        nc.gpsimd.dma_start(o[pending_store[0]], pending_store[1][:])

    nc.compile()
    return nc


_NC = None


def _get_nc():
    global _NC
    if _NC is None:
        _NC = build()
    return _NC


def _to_pmajor(block: np.ndarray) -> np.ndarray:
    # (512, 512) row-major -> (128, 4, 512): row r = t*P + p lands at
    # [p, t, :], so every SBUF partition's bytes are contiguous in DRAM.
    return block.reshape(T, P, B).transpose(1, 0, 2)


def make_in_maps(sims: np.ndarray) -> list[dict[str, np.ndarray]]:
    in_maps = []
    for c in range(NCORES):
        ab_stack = np.empty((NOFF, P, 2, T, B), np.float16)
        ad_stack = np.empty((NDIAG, P, T, B), np.float16)
        for k, (i, j) in enumerate(CORE_PAIRS[c]):
            if k < NOFF:
                assert i != j
                ab_stack[k, :, 0] = _to_pmajor(
                    sims[i * B:(i + 1) * B, j * B:(j + 1) * B]).astype(
                        np.float16)
                ab_stack[k, :, 1] = _to_pmajor(
                    np.ascontiguousarray(
                        sims[j * B:(j + 1) * B, i * B:(i + 1) * B].T)).astype(
                            np.float16)
            else:
                assert i == j
                a = sims[i * B:(i + 1) * B, i * B:(i + 1) * B].copy()
                np.fill_diagonal(a, MASK)
                ad_stack[k - NOFF] = _to_pmajor(a).astype(np.float16)
        in_maps.append({"ab": ab_stack, "ad": ad_stack})
    return in_maps


def assemble(results: list[dict[str, np.ndarray]]) -> np.ndarray:
    out = np.empty((N, N), np.float32)
    for c in range(NCORES):
        o_pm = results[c]["o"]  # (NSLOTS, P, T, B) fp16, partition-major
        o_stack = o_pm.astype(np.float32).transpose(0, 2, 1, 3).reshape(
            NSLOTS, B, B)
        for k, (i, j) in enumerate(CORE_PAIRS[c]):
            out[i * B:(i + 1) * B, j * B:(j + 1) * B] = o_stack[k]
            if i != j:
                out[j * B:(j + 1) * B, i * B:(i + 1) * B] = o_stack[k].T
    return out


def run_on_hw(sims: np.ndarray, **spmd_kwargs):
    """Run the kernel on the 8 NeuronCores. Returns (out, BassKernelResults).

    The device occasionally throws a transient NRT_EXEC_UNIT_UNRECOVERABLE
    and needs ~a minute to come back, so failed runs are retried."""
    import time

    nc = _get_nc()
    in_maps = make_in_maps(sims)
    last_exc = None
    for attempt in range(3):
        if attempt:
            time.sleep(75)
        try:
            res = run_bass_kernel_spmd(
                nc, in_maps, core_ids=list(range(NCORES)), **spmd_kwargs
            )
            return assemble(res.results), res
        except Exception as exc:  # noqa: BLE001 - device flake, retry
            last_exc = exc
    raise last_exc


def kernel(similarities: np.ndarray) -> np.ndarray:
    sims = np.ascontiguousarray(similarities, dtype=np.float32)
    assert sims.shape == (N, N)
    out, _ = run_on_hw(sims)
    return out


if __name__ == "__main__":
    rng = np.random.default_rng(0)
    sims = rng.standard_normal((N, N), dtype=np.float32)
    out = kernel(similarities=sims)
    print("out", out.shape, out.dtype, float(out.max()))


# revision 9
# speedup vs baseline: 3.2600x; 1.0065x over previous
"""Trainium2 Bass kernel for nn_BestHits: out = bh * bh.T where
bh = blockwise-softmax(mask_diag(similarities) / TAU) over 256-wide column groups.

Strategy: out is symmetric (out.T = bh.T * bh = out), so only the upper
triangle of 512x512 block-pairs is computed on device. The 16x16 block grid
has 136 upper-incl-diagonal pairs = 17 per core on 8 cores (each core gets
exactly 2 diagonal + 15 off-diagonal pairs -> perfectly uniform SPMD work).
B-side blocks are staged pre-transposed by the host (layout-only, free).

v3 (measured-rate driven; v1 was 142.6us with ACT 113.6/DVE 111/DMA 104.7
walls; v2's tensor_scalar+accum experiment measured: TT/TS at 2x with
all-16-bit packed operands, accum-TS stuck at 1x + READ_ACCUMULATOR,
GpSimd TT at ~2.1ns/elem):

  * Inputs staged fp16 on the host (free): 16 MiB loads/core vs 32.
  * One merged [P, side, t-pair, B] exp per t-pair covers BOTH the A and
    BT halves in a single big ACTIVATE (2 per off slot, 1137ns/1024e rate)
    with bias=-30 folded in: exp(x/TAU - 30) rescales both softmax
    numerator and denominator consistently (out invariant) and keeps
    W = za*zbt below bf16 overflow for unclamped N(0,1) inputs.
  * A-side group sums as a 2-level bf16 pairwise tree (two 2x
    tensor_tensor adds) + one 1x tensor_reduce over the last 64: ~1.5us
    vs 2.2us flat reduce, vs 3.9us accum-TS, vs 5.4us ACT-accum.
  * Product out = (za*zbt) * (ra x rp): W = za*zbt (2x TT), scale tile
    S[p,t,c] = ra[p,t,g(c)]*rp_{g(t)}[c] built on GpSimd as 8 small
    tensor_scalar_muls (f32 rp row * f32 ra per-partition scalar -> bf16),
    final = W*S (2x TT). DVE does 2 passes at 0.52ns/elem instead of
    8 1x scalar_tensor_tensors.
  * Diagonal slots: ra-apply (wa) moved to ACT (Copy activation with
    per-partition scale), PE transpose path unchanged.
  * Stores on the GpSimd DMA ring, deferred one slot.

Per-slot engine budget (off): ACT 4.0us, DVE ~5.5us, GpSimd ~4.9us,
PE ~2.5us, DMA ~3.7us -> projected walls DVE ~91us, others below.

Per-core HBM traffic: 15*1 MiB + 2*0.5 MiB loads + 17*0.5 MiB stores
= 24.5 MiB.
"""
import sys

import numpy as np

sys.path.insert(0, "/opt/trn_rl_repo")

from contextlib import ExitStack

import concourse.bass as bass  # noqa: F401  (registers AP machinery)
import concourse.tile as tile
from concourse import bacc, masks, mybir
from concourse.bass_utils import run_bass_kernel_spmd

N = 8192          # full matrix side
B = 512           # block side
NB = N // B       # 16 blocks per side
P = 128           # SBUF partitions
T = B // P        # 4 row-subtiles per block
GRP = 256         # softmax group width
NG = B // GRP     # 2 groups per block side
TAU = 0.1
NDIAG = 2         # diagonal pairs per core (the last NDIAG slots)
NSLOTS = 17       # block-pairs per core
NOFF = NSLOTS - NDIAG
NCORES = 8
MASK = -60000.0   # pre-masked diagonal value (fp16-representable; exp->0)
EXP_BIAS = -30.0  # exp(x/TAU + EXP_BIAS): overflow headroom for za*zbt

F32 = mybir.dt.float32
F16 = mybir.dt.float16
BF16 = mybir.dt.bfloat16

AF = mybir.ActivationFunctionType
OP = mybir.AluOpType


def core_pairs() -> list[list[tuple[int, int]]]:
    """136 upper-triangle block pairs distributed 17-per-core; the 2 diagonal
    pairs of each core come last (the kernel treats those slots specially)."""
    diag = [(i, i) for i in range(NB)]
    off = [(i, j) for i in range(NB) for j in range(i + 1, NB)]
    cps: list[list[tuple[int, int]]] = [[] for _ in range(NCORES)]
    for idx, p in enumerate(off):
        cps[idx % NCORES].append(p)
    for idx, p in enumerate(diag):
        cps[idx % NCORES].append(p)
    return cps


CORE_PAIRS = core_pairs()


def build():
    """Build + compile the (single-program, 8-core SPMD) Bass kernel."""
    nc = bacc.Bacc(
        "TRN2",
        target_bir_lowering=False,
        debug=False,
        enable_asserts=True,
        num_devices=NCORES,
    )
    ab = nc.dram_tensor("ab", [NOFF, P, 2, T, B], F16, kind="ExternalInput").ap()
    ad = nc.dram_tensor("ad", [NDIAG, P, T, B], F16, kind="ExternalInput").ap()
    o = nc.dram_tensor("o", [NSLOTS, P, T, B], F16, kind="ExternalOutput").ap()

    with tile.TileContext(nc) as tc, ExitStack() as ctx:
        const_pool = ctx.enter_context(tc.tile_pool(name="const", bufs=1))
        ident = const_pool.tile([P, P], BF16)
        masks.make_identity(nc, ident[:])
        # All-ones stationary: one matmul both colsums zbt's partition groups
        # AND broadcasts the result to all 128 PSUM partitions. bf16 so the
        # matmuls run in one pass (fp32 matmul = 2 passes).
        ones_mat = const_pool.tile([P, P], BF16)
        nc.gpsimd.memset(ones_mat[:], 1.0)
        bias_sb = const_pool.tile([P, 1], F32)
        nc.gpsimd.memset(bias_sb[:], EXP_BIAS)

        ab_pool = ctx.enter_context(tc.tile_pool(name="ab_sb", bufs=5))
        ad_pool = ctx.enter_context(tc.tile_pool(name="ad_sb", bufs=2))
        z_pool = ctx.enter_context(tc.tile_pool(name="zab", bufs=4))
        w_pool = ctx.enter_context(tc.tile_pool(name="w", bufs=4))
        s_pool = ctx.enter_context(tc.tile_pool(name="s", bufs=4))
        h_pool = ctx.enter_context(tc.tile_pool(name="h", bufs=4))
        o_pool = ctx.enter_context(tc.tile_pool(name="o_sb", bufs=4))
        st_pool = ctx.enter_context(tc.tile_pool(name="st", bufs=10))
        rp_pool = ctx.enter_context(tc.tile_pool(name="rp", bufs=4))
        dg_pool = ctx.enter_context(tc.tile_pool(name="dg", bufs=2))
        ps_pool = ctx.enter_context(tc.tile_pool(name="ps", bufs=3, space="PSUM"))

        def tree_sums(za, sa, ra):
            """sa[p, t, g] = sum_c za[p, t, g*256+c]; ra = 1/sa.
            Two 2x-mode bf16 pairwise-add stages + one small 1x reduce."""
            za4 = za.rearrange("p t (g c) -> p (t g) c", c=GRP)
            h1 = h_pool.tile([P, T * NG, GRP // 2], BF16)
            h2 = h_pool.tile([P, T * NG, GRP // 4], BF16)
            nc.vector.tensor_tensor(h1[:], za4[:, :, 0:128], za4[:, :, 128:256],
                                    op=OP.add)
            nc.vector.tensor_tensor(h2[:], h1[:, :, 0:64], h1[:, :, 64:128],
                                    op=OP.add)
            nc.vector.tensor_reduce(sa.rearrange("p t g -> p (t g)"), h2[:],
                                    axis=mybir.AxisListType.X, op=OP.add)
            nc.vector.reciprocal(ra.rearrange("p t g -> p (t g)"),
                                 sa.rearrange("p t g -> p (t g)"))

        # Diagonal slots are interleaved mid-program: their short chains give
        # ACT/DVE low-dependency filler work between full off-slot chains.
        order = [*range(0, 7), NOFF, *range(7, 12), NOFF + 1, *range(12, NOFF)]
        # Stores are deferred one slot: issued immediately, store(k) sits at
        # the GpSimd queue head waiting on slot k's full product and blocks
        # slot k+1's work behind it (head-of-line serialization).
        pending_store = None
        for k in order:
            diag_slot = k >= NOFF
            if not diag_slot:
                # --- off-diagonal pair: A and host-pre-transposed B ---
                ab_sb = ab_pool.tile([P, 2, T, B], F16)
                nc.sync.dma_start(ab_sb[:], ab[k])

                # Separate za/zbt tiles: DVE reads za while GpSimd/PE read
                # zbt -- a merged tile measured 2.4x slower DVE tensor_scalars
                # (SBUF bank contention). BT exp split so PE starts early.
                zbt_t = z_pool.tile([P, T, B], BF16, name="zbt")
                za_t = z_pool.tile([P, T, B], BF16, name="za")
                za = za_t[:]
                zbt = zbt_t[:]
                s_ps = ps_pool.tile([P, NG, B], F32, name="p23")
                for g in range(NG):
                    ts = slice(NG * g, NG * (g + 1))
                    nc.scalar.activation(zbt_t[:, ts, :], ab_sb[:, 1, ts, :],
                                         AF.Exp, scale=1.0 / TAU,
                                         bias=bias_sb[:])
                    for u in range(NG):
                        nc.tensor.matmul(
                            s_ps[:, g, :], ones_mat[:], zbt[:, g * NG + u, :],
                            start=(u == 0), stop=(u == NG - 1),
                        )
                for h in range(NG):
                    ts = slice(NG * h, NG * (h + 1))
                    nc.scalar.activation(za_t[:, ts, :], ab_sb[:, 0, ts, :],
                                         AF.Exp, scale=1.0 / TAU,
                                         bias=bias_sb[:])
                rp_sb = rp_pool.tile([P, NG, B], F32)
                nc.vector.reciprocal_approx_fast(
                    rp_sb[:].rearrange("p g b -> p (g b)"),
                    s_ps[:].rearrange("p g b -> p (g b)"))

                # A side: group sums + ra.
                sa = st_pool.tile([P, T, NG], F32, name="sa")
                ra = st_pool.tile([P, T, NG], F32, name="ra")
                tree_sums(za, sa[:], ra[:])

                # X = bhB.T in fp16, fully on GpSimd tensor_tensors (2 ops,
                # rp broadcast across the t-pair; GpSimd TT measured
                # 2.25ns/elem -- its tensor_scalar is 7x slower, avoid).
                x_sb = s_pool.tile([P, T, B], F16)
                for h in range(NG):
                    ts = slice(NG * h, NG * (h + 1))
                    nc.gpsimd.tensor_mul(
                        x_sb[:, ts, :], zbt[:, ts, :],
                        rp_sb[:, h:h + 1, :].broadcast_to([P, NG, B]),
                    )

                # wa = za*ra via 2x-mode tensor_scalar_mul (253ns/region
                # measured), then ONE 2x-mode tensor_tensor for out = wa*X.
                wa = w_pool.tile([P, T, B], BF16)
                o_sb = o_pool.tile([P, T, B], F16)
                for t in range(T):
                    for g in range(NG):
                        cs = slice(g * GRP, (g + 1) * GRP)
                        nc.vector.tensor_scalar_mul(
                            wa[:, t, cs], za[:, t, cs], ra[:, t, g:g + 1])
                nc.vector.tensor_tensor(o_sb[:], wa[:], x_sb[:], op=OP.mult)
            else:
                # --- diagonal pair: B == A, PE bf16 transpose ---
                a_sb = ad_pool.tile([P, T, B], F16)
                nc.sync.dma_start(a_sb[:], ad[k - NOFF])
                zad = z_pool.tile([P, T, B], BF16)
                for h in range(NG):
                    ts = slice(NG * h, NG * (h + 1))
                    nc.scalar.activation(zad[:, ts, :], a_sb[:, ts, :],
                                         AF.Exp, scale=1.0 / TAU,
                                         bias=bias_sb[:])
                sa = st_pool.tile([P, T, NG], F32, name="sa")
                ra = st_pool.tile([P, T, NG], F32, name="ra")
                tree_sums(zad[:], sa[:], ra[:])
                wa = w_pool.tile([P, T, B], BF16)
                for t in range(T):
                    for g in range(NG):
                        cs = slice(g * GRP, (g + 1) * GRP)
                        nc.vector.tensor_scalar_mul(
                            wa[:, t, cs], zad[:, t, cs], ra[:, t, g:g + 1])
                dg = dg_pool.tile([P, T * NG, P], BF16)
                nc.gpsimd.tensor_mul(
                    dg[:],
                    ident[:].rearrange("p (one c) -> p one c", one=1)
                    .broadcast_to([P, T * NG, P]),
                    ra[:].rearrange("p t g -> p (t g)")
                    .rearrange("p (tg one) -> p tg one", one=1)
                    .broadcast_to([P, T * NG, P]),
                )
                # Two v-waves through one 2-bank PSUM tile; wave 2 reuses the
                # banks after wave 1's products are read.
                p23 = ps_pool.tile([P, NG, B], F32, name="p23")
                o_sb = o_pool.tile([P, T, B], F16)
                for w in range(NG):
                    for hv in range(NG):
                        v = w * NG + hv
                        for u in range(T):
                            nc.tensor.matmul(
                                p23[:, hv, u * P:(u + 1) * P],
                                zad[:, u, v * P:(v + 1) * P],
                                dg[:, u * NG + (v // NG), :],
                            )
                    nc.vector.tensor_tensor(
                        o_sb[:, w * NG:(w + 1) * NG, :],
                        wa[:, w * NG:(w + 1) * NG, :], p23[:],
                        op=OP.mult)

            # One whole-block store per slot on the SWDGE (gpsimd) ring: it
            # never queues ahead of loads on the sync HWDGE ring.
            if pending_store is not None:
                nc.gpsimd.dma_start(o[pending_store[0]], pending_store[1][:])
            pending_store = (k, o_sb)
        nc.gpsimd.dma_start(o[pending_store[0]], pending_store[1][:])

    nc.compile()
    return nc


_NC = None


def _get_nc():
    global _NC
    if _NC is None:
        _NC = build()
    return _NC


def _to_pmajor(block: np.ndarray) -> np.ndarray:
    # (512, 512) row-major -> (128, 4, 512): row r = t*P + p lands at
    # [p, t, :], so every SBUF partition's bytes are contiguous in DRAM.
    return block.reshape(T, P, B).transpose(1, 0, 2)


def make_in_maps(sims: np.ndarray) -> list[dict[str, np.ndarray]]:
    in_maps = []
    for c in range(NCORES):
        ab_stack = np.empty((NOFF, P, 2, T, B), np.float16)
        ad_stack = np.empty((NDIAG, P, T, B), np.float16)
        for k, (i, j) in enumerate(CORE_PAIRS[c]):
            if k < NOFF:
                assert i != j
                ab_stack[k, :, 0] = _to_pmajor(
                    sims[i * B:(i + 1) * B, j * B:(j + 1) * B]).astype(
                        np.float16)
                ab_stack[k, :, 1] = _to_pmajor(
                    np.ascontiguousarray(
                        sims[j * B:(j + 1) * B, i * B:(i + 1) * B].T)).astype(
                            np.float16)
            else:
                assert i == j
                a = sims[i * B:(i + 1) * B, i * B:(i + 1) * B].copy()
                np.fill_diagonal(a, MASK)
                ad_stack[k - NOFF] = _to_pmajor(a).astype(np.float16)
        in_maps.append({"ab": ab_stack, "ad": ad_stack})
    return in_maps


def assemble(results: list[dict[str, np.ndarray]]) -> np.ndarray:
    out = np.empty((N, N), np.float32)
    for c in range(NCORES):
        o_pm = results[c]["o"]  # (NSLOTS, P, T, B) fp16, partition-major
        o_stack = o_pm.astype(np.float32).transpose(0, 2, 1, 3).reshape(
            NSLOTS, B, B)
        for k, (i, j) in enumerate(CORE_PAIRS[c]):
            out[i * B:(i + 1) * B, j * B:(j + 1) * B] = o_stack[k]
            if i != j:
                out[j * B:(j + 1) * B, i * B:(i + 1) * B] = o_stack[k].T
    return out


def run_on_hw(sims: np.ndarray, **spmd_kwargs):
    """Run the kernel on the 8 NeuronCores. Returns (out, BassKernelResults).

    The device occasionally throws a transient NRT_EXEC_UNIT_UNRECOVERABLE
    and needs ~a minute to come back, so failed runs are retried."""
    import time

    nc = _get_nc()
    in_maps = make_in_maps(sims)
    last_exc = None
    for attempt in range(3):
        if attempt:
            time.sleep(75)
        try:
            res = run_bass_kernel_spmd(
                nc, in_maps, core_ids=list(range(NCORES)), **spmd_kwargs
            )
            return assemble(res.results), res
        except Exception as exc:  # noqa: BLE001 - device flake, retry
            last_exc = exc
    raise last_exc


def kernel(similarities: np.ndarray) -> np.ndarray:
    sims = np.ascontiguousarray(similarities, dtype=np.float32)
    assert sims.shape == (N, N)
    out, _ = run_on_hw(sims)
    return out


if __name__ == "__main__":
    rng = np.random.default_rng(0)
    sims = rng.standard_normal((N, N), dtype=np.float32)
    out = kernel(similarities=sims)
    print("out", out.shape, out.dtype, float(out.max()))


# revision 10
# speedup vs baseline: 3.2845x; 1.0075x over previous
"""Trainium2 Bass kernel for nn_BestHits: out = bh * bh.T where
bh = blockwise-softmax(mask_diag(similarities) / TAU) over 256-wide column groups.

Strategy: out is symmetric (out.T = bh.T * bh = out), so only the upper
triangle of 512x512 block-pairs is computed on device. The 16x16 block grid
has 136 upper-incl-diagonal pairs = 17 per core on 8 cores (each core gets
exactly 2 diagonal + 15 off-diagonal pairs -> perfectly uniform SPMD work).
B-side blocks are staged pre-transposed by the host (layout-only, free).

v3 (measured-rate driven; v1 was 142.6us with ACT 113.6/DVE 111/DMA 104.7
walls; v2's tensor_scalar+accum experiment measured: TT/TS at 2x with
all-16-bit packed operands, accum-TS stuck at 1x + READ_ACCUMULATOR,
GpSimd TT at ~2.1ns/elem):

  * Inputs staged fp16 on the host (free): 16 MiB loads/core vs 32.
  * One merged [P, side, t-pair, B] exp per t-pair covers BOTH the A and
    BT halves in a single big ACTIVATE (2 per off slot, 1137ns/1024e rate)
    with bias=-30 folded in: exp(x/TAU - 30) rescales both softmax
    numerator and denominator consistently (out invariant) and keeps
    W = za*zbt below bf16 overflow for unclamped N(0,1) inputs.
  * A-side group sums as a 2-level bf16 pairwise tree (two 2x
    tensor_tensor adds) + one 1x tensor_reduce over the last 64: ~1.5us
    vs 2.2us flat reduce, vs 3.9us accum-TS, vs 5.4us ACT-accum.
  * Product out = (za*zbt) * (ra x rp): W = za*zbt (2x TT), scale tile
    S[p,t,c] = ra[p,t,g(c)]*rp_{g(t)}[c] built on GpSimd as 8 small
    tensor_scalar_muls (f32 rp row * f32 ra per-partition scalar -> bf16),
    final = W*S (2x TT). DVE does 2 passes at 0.52ns/elem instead of
    8 1x scalar_tensor_tensors.
  * Diagonal slots: ra-apply (wa) moved to ACT (Copy activation with
    per-partition scale), PE transpose path unchanged.
  * Stores on the GpSimd DMA ring, deferred one slot.

Per-slot engine budget (off): ACT 4.0us, DVE ~5.5us, GpSimd ~4.9us,
PE ~2.5us, DMA ~3.7us -> projected walls DVE ~91us, others below.

Per-core HBM traffic: 15*1 MiB + 2*0.5 MiB loads + 17*0.5 MiB stores
= 24.5 MiB.
"""
import sys

import numpy as np

sys.path.insert(0, "/opt/trn_rl_repo")

from contextlib import ExitStack

import concourse.bass as bass  # noqa: F401  (registers AP machinery)
import concourse.tile as tile
from concourse import bacc, masks, mybir
from concourse.bass_utils import run_bass_kernel_spmd

N = 8192          # full matrix side
B = 512           # block side
NB = N // B       # 16 blocks per side
P = 128           # SBUF partitions
T = B // P        # 4 row-subtiles per block
GRP = 256         # softmax group width
NG = B // GRP     # 2 groups per block side
TAU = 0.1
NDIAG = 2         # diagonal pairs per core (the last NDIAG slots)
NSLOTS = 17       # block-pairs per core
NOFF = NSLOTS - NDIAG
NCORES = 8
MASK = -60000.0   # pre-masked diagonal value (fp16-representable; exp->0)
EXP_BIAS = -30.0  # exp(x/TAU + EXP_BIAS): overflow headroom for za*zbt

F32 = mybir.dt.float32
F16 = mybir.dt.float16
BF16 = mybir.dt.bfloat16

AF = mybir.ActivationFunctionType
OP = mybir.AluOpType


def core_pairs() -> list[list[tuple[int, int]]]:
    """136 upper-triangle block pairs distributed 17-per-core; the 2 diagonal
    pairs of each core come last (the kernel treats those slots specially)."""
    diag = [(i, i) for i in range(NB)]
    off = [(i, j) for i in range(NB) for j in range(i + 1, NB)]
    cps: list[list[tuple[int, int]]] = [[] for _ in range(NCORES)]
    for idx, p in enumerate(off):
        cps[idx % NCORES].append(p)
    for idx, p in enumerate(diag):
        cps[idx % NCORES].append(p)
    return cps


CORE_PAIRS = core_pairs()


def build():
    """Build + compile the (single-program, 8-core SPMD) Bass kernel."""
    nc = bacc.Bacc(
        "TRN2",
        target_bir_lowering=False,
        debug=False,
        enable_asserts=True,
        num_devices=NCORES,
    )
    ab = nc.dram_tensor("ab", [NOFF, P, 2, T, B], F16, kind="ExternalInput").ap()
    ad = nc.dram_tensor("ad", [NDIAG, P, T, B], F16, kind="ExternalInput").ap()
    o = nc.dram_tensor("o", [NSLOTS, P, T, B], F16, kind="ExternalOutput").ap()

    with tile.TileContext(nc) as tc, ExitStack() as ctx:
        const_pool = ctx.enter_context(tc.tile_pool(name="const", bufs=1))
        ident = const_pool.tile([P, P], BF16)
        masks.make_identity(nc, ident[:])
        # All-ones stationary: one matmul both colsums zbt's partition groups
        # AND broadcasts the result to all 128 PSUM partitions. bf16 so the
        # matmuls run in one pass (fp32 matmul = 2 passes).
        ones_mat = const_pool.tile([P, P], BF16)
        nc.gpsimd.memset(ones_mat[:], 1.0)
        bias_sb = const_pool.tile([P, 1], F32)
        nc.gpsimd.memset(bias_sb[:], EXP_BIAS)

        ab_pool = ctx.enter_context(tc.tile_pool(name="ab_sb", bufs=5))
        ad_pool = ctx.enter_context(tc.tile_pool(name="ad_sb", bufs=2))
        z_pool = ctx.enter_context(tc.tile_pool(name="zab", bufs=6))
        w_pool = ctx.enter_context(tc.tile_pool(name="w", bufs=4))
        s_pool = ctx.enter_context(tc.tile_pool(name="s", bufs=4))
        h_pool = ctx.enter_context(tc.tile_pool(name="h", bufs=4))
        o_pool = ctx.enter_context(tc.tile_pool(name="o_sb", bufs=4))
        st_pool = ctx.enter_context(tc.tile_pool(name="st", bufs=10))
        rp_pool = ctx.enter_context(tc.tile_pool(name="rp", bufs=4))
        dg_pool = ctx.enter_context(tc.tile_pool(name="dg", bufs=2))
        ps_pool = ctx.enter_context(tc.tile_pool(name="ps", bufs=4, space="PSUM"))

        def tree_sums(za, sa, ra):
            """sa[p, t, g] = sum_c za[p, t, g*256+c]; ra = 1/sa.
            Two 2x-mode bf16 pairwise-add stages + one small 1x reduce,
            all through one scratch tile (fewer pool alloc/release syncs)."""
            za4 = za.rearrange("p t (g c) -> p (t g) c", c=GRP)
            h = h_pool.tile([P, T * NG, GRP // 2 + GRP // 4], BF16)
            h1 = h[:, :, 0:128]
            h2 = h[:, :, 128:192]
            nc.vector.tensor_tensor(h1, za4[:, :, 0:128], za4[:, :, 128:256],
                                    op=OP.add)
            nc.vector.tensor_tensor(h2, h1[:, :, 0:64], h1[:, :, 64:128],
                                    op=OP.add)
            nc.vector.tensor_reduce(sa.rearrange("p t g -> p (t g)"), h2,
                                    axis=mybir.AxisListType.X, op=OP.add)
            nc.vector.reciprocal(ra.rearrange("p t g -> p (t g)"),
                                 sa.rearrange("p t g -> p (t g)"))

        # Diagonal slots are interleaved mid-program: their short chains give
        # ACT/DVE low-dependency filler work between full off-slot chains.
        order = [*range(0, 7), NOFF, *range(7, 12), NOFF + 1, *range(12, NOFF)]
        # Stores are deferred one slot: issued immediately, store(k) sits at
        # the GpSimd queue head waiting on slot k's full product and blocks
        # slot k+1's work behind it (head-of-line serialization).
        pending_store = None
        for k in order:
            diag_slot = k >= NOFF
            if not diag_slot:
                # --- off-diagonal pair: A and host-pre-transposed B ---
                ab_sb = ab_pool.tile([P, 2, T, B], F16)
                nc.sync.dma_start(ab_sb[:], ab[k])

                # Separate za/zbt tiles: DVE reads za while GpSimd/PE read
                # zbt -- a merged tile measured 2.4x slower DVE tensor_scalars
                # (SBUF bank contention). BT exp split so PE starts early.
                zbt_t = z_pool.tile([P, T, B], BF16, name="zbt")
                za_t = z_pool.tile([P, T, B], BF16, name="za")
                za = za_t[:]
                zbt = zbt_t[:]
                s_ps = ps_pool.tile([P, NG, B], F32, name="p23")
                for g in range(NG):
                    ts = slice(NG * g, NG * (g + 1))
                    nc.scalar.activation(zbt_t[:, ts, :], ab_sb[:, 1, ts, :],
                                         AF.Exp, scale=1.0 / TAU,
                                         bias=bias_sb[:])
                    for u in range(NG):
                        nc.tensor.matmul(
                            s_ps[:, g, :], ones_mat[:], zbt[:, g * NG + u, :],
                            start=(u == 0), stop=(u == NG - 1),
                        )
                for h in range(NG):
                    ts = slice(NG * h, NG * (h + 1))
                    nc.scalar.activation(za_t[:, ts, :], ab_sb[:, 0, ts, :],
                                         AF.Exp, scale=1.0 / TAU,
                                         bias=bias_sb[:])
                rp_sb = rp_pool.tile([P, NG, B], F32)
                nc.vector.reciprocal_approx_fast(
                    rp_sb[:].rearrange("p g b -> p (g b)"),
                    s_ps[:].rearrange("p g b -> p (g b)"))

                # A side: group sums + ra.
                sr = st_pool.tile([P, 2, T, NG], F32, name="sr")
                sa, ra = sr[:, 0], sr[:, 1]
                tree_sums(za, sa, ra)

                # X = bhB.T in fp16, fully on GpSimd tensor_tensors (2 ops,
                # rp broadcast across the t-pair; GpSimd TT measured
                # 2.25ns/elem -- its tensor_scalar is 7x slower, avoid).
                x_sb = s_pool.tile([P, T, B], BF16)
                for h in range(NG):
                    ts = slice(NG * h, NG * (h + 1))
                    nc.gpsimd.tensor_mul(
                        x_sb[:, ts, :], zbt[:, ts, :],
                        rp_sb[:, h:h + 1, :].broadcast_to([P, NG, B]),
                    )

                # wa = za*ra via 2x-mode tensor_scalar_mul (253ns/region
                # measured), then ONE 2x-mode tensor_tensor for out = wa*X.
                wa = w_pool.tile([P, T, B], BF16)
                o_sb = o_pool.tile([P, T, B], F16)
                for t in range(T):
                    for g in range(NG):
                        cs = slice(g * GRP, (g + 1) * GRP)
                        nc.vector.tensor_scalar_mul(
                            wa[:, t, cs], za[:, t, cs], ra[:, t, g:g + 1])
                nc.vector.tensor_tensor(o_sb[:], wa[:], x_sb[:], op=OP.mult)
            else:
                # --- diagonal pair: B == A, PE bf16 transpose ---
                a_sb = ad_pool.tile([P, T, B], F16)
                nc.sync.dma_start(a_sb[:], ad[k - NOFF])
                zad = z_pool.tile([P, T, B], BF16)
                for h in range(NG):
                    ts = slice(NG * h, NG * (h + 1))
                    nc.scalar.activation(zad[:, ts, :], a_sb[:, ts, :],
                                         AF.Exp, scale=1.0 / TAU,
                                         bias=bias_sb[:])
                sr = st_pool.tile([P, 2, T, NG], F32, name="sr")
                sa, ra = sr[:, 0], sr[:, 1]
                tree_sums(zad[:], sa, ra)
                wa = w_pool.tile([P, T, B], BF16)
                for t in range(T):
                    for g in range(NG):
                        cs = slice(g * GRP, (g + 1) * GRP)
                        nc.vector.tensor_scalar_mul(
                            wa[:, t, cs], zad[:, t, cs], ra[:, t, g:g + 1])
                dg = dg_pool.tile([P, T * NG, P], BF16)
                nc.gpsimd.tensor_mul(
                    dg[:],
                    ident[:].rearrange("p (one c) -> p one c", one=1)
                    .broadcast_to([P, T * NG, P]),
                    ra[:].rearrange("p t g -> p (t g)")
                    .rearrange("p (tg one) -> p tg one", one=1)
                    .broadcast_to([P, T * NG, P]),
                )
                # Two v-waves through one 2-bank PSUM tile; wave 2 reuses the
                # banks after wave 1's products are read.
                p23 = ps_pool.tile([P, NG, B], F32, name="p23")
                o_sb = o_pool.tile([P, T, B], F16)
                for w in range(NG):
                    for hv in range(NG):
                        v = w * NG + hv
                        for u in range(T):
                            nc.tensor.matmul(
                                p23[:, hv, u * P:(u + 1) * P],
                                zad[:, u, v * P:(v + 1) * P],
                                dg[:, u * NG + (v // NG), :],
                            )
                    nc.vector.tensor_tensor(
                        o_sb[:, w * NG:(w + 1) * NG, :],
                        wa[:, w * NG:(w + 1) * NG, :], p23[:],
                        op=OP.mult)

            # One whole-block store per slot on the SWDGE (gpsimd) ring: it
            # never queues ahead of loads on the sync HWDGE ring.
            if pending_store is not None:
                eng = nc.sync if pending_store[0] % 3 == 2 else nc.gpsimd
                eng.dma_start(o[pending_store[0]], pending_store[1][:])
            pending_store = (k, o_sb)
        eng = nc.sync if pending_store[0] % 3 == 2 else nc.gpsimd
        eng.dma_start(o[pending_store[0]], pending_store[1][:])

    nc.compile()
    return nc


_NC = None


def _get_nc():
    global _NC
    if _NC is None:
        _NC = build()
    return _NC


def _to_pmajor(block: np.ndarray) -> np.ndarray:
    # (512, 512) row-major -> (128, 4, 512): row r = t*P + p lands at
    # [p, t, :], so every SBUF partition's bytes are contiguous in DRAM.
    return block.reshape(T, P, B).transpose(1, 0, 2)


def make_in_maps(sims: np.ndarray) -> list[dict[str, np.ndarray]]:
    in_maps = []
    for c in range(NCORES):
        ab_stack = np.empty((NOFF, P, 2, T, B), np.float16)
        ad_stack = np.empty((NDIAG, P, T, B), np.float16)
        for k, (i, j) in enumerate(CORE_PAIRS[c]):
            if k < NOFF:
                assert i != j
                ab_stack[k, :, 0] = _to_pmajor(
                    sims[i * B:(i + 1) * B, j * B:(j + 1) * B]).astype(
                        np.float16)
                ab_stack[k, :, 1] = _to_pmajor(
                    np.ascontiguousarray(
                        sims[j * B:(j + 1) * B, i * B:(i + 1) * B].T)).astype(
                            np.float16)
            else:
                assert i == j
                a = sims[i * B:(i + 1) * B, i * B:(i + 1) * B].copy()
                np.fill_diagonal(a, MASK)
                ad_stack[k - NOFF] = _to_pmajor(a).astype(np.float16)
        in_maps.append({"ab": ab_stack, "ad": ad_stack})
    return in_maps


def assemble(results: list[dict[str, np.ndarray]]) -> np.ndarray:
    out = np.empty((N, N), np.float32)
    for c in range(NCORES):
        o_pm = results[c]["o"]  # (NSLOTS, P, T, B) fp16, partition-major
        o_stack = o_pm.astype(np.float32).transpose(0, 2, 1, 3).reshape(
            NSLOTS, B, B)
        for k, (i, j) in enumerate(CORE_PAIRS[c]):
            out[i * B:(i + 1) * B, j * B:(j + 1) * B] = o_stack[k]
            if i != j:
                out[j * B:(j + 1) * B, i * B:(i + 1) * B] = o_stack[k].T
    return out


def run_on_hw(sims: np.ndarray, **spmd_kwargs):
    """Run the kernel on the 8 NeuronCores. Returns (out, BassKernelResults).

    The device occasionally throws a transient NRT_EXEC_UNIT_UNRECOVERABLE
    and needs ~a minute to come back, so failed runs are retried."""
    import time

    nc = _get_nc()
    in_maps = make_in_maps(sims)
    last_exc = None
    for attempt in range(3):
        if attempt:
            time.sleep(75)
        try:
            res = run_bass_kernel_spmd(
                nc, in_maps, core_ids=list(range(NCORES)), **spmd_kwargs
            )
            return assemble(res.results), res
        except Exception as exc:  # noqa: BLE001 - device flake, retry
            last_exc = exc
    raise last_exc


def kernel(similarities: np.ndarray) -> np.ndarray:
    sims = np.ascontiguousarray(similarities, dtype=np.float32)
    assert sims.shape == (N, N)
    out, _ = run_on_hw(sims)
    return out


if __name__ == "__main__":
    rng = np.random.default_rng(0)
    sims = rng.standard_normal((N, N), dtype=np.float32)
    out = kernel(similarities=sims)
    print("out", out.shape, out.dtype, float(out.max()))


# revision 11
# speedup vs baseline: 3.2904x; 1.0018x over previous
"""Trainium2 Bass kernel for nn_BestHits: out = bh * bh.T where
bh = blockwise-softmax(mask_diag(similarities) / TAU) over 256-wide column groups.

Strategy: out is symmetric (out.T = bh.T * bh = out), so only the upper
triangle of 512x512 block-pairs is computed on device. The 16x16 block grid
has 136 upper-incl-diagonal pairs = 17 per core on 8 cores (each core gets
exactly 2 diagonal + 15 off-diagonal pairs -> perfectly uniform SPMD work).
B-side blocks are staged pre-transposed by the host (layout-only, free).

v3 (measured-rate driven; v1 was 142.6us with ACT 113.6/DVE 111/DMA 104.7
walls; v2's tensor_scalar+accum experiment measured: TT/TS at 2x with
all-16-bit packed operands, accum-TS stuck at 1x + READ_ACCUMULATOR,
GpSimd TT at ~2.1ns/elem):

  * Inputs staged fp16 on the host (free): 16 MiB loads/core vs 32.
  * One merged [P, side, t-pair, B] exp per t-pair covers BOTH the A and
    BT halves in a single big ACTIVATE (2 per off slot, 1137ns/1024e rate)
    with bias=-30 folded in: exp(x/TAU - 30) rescales both softmax
    numerator and denominator consistently (out invariant) and keeps
    W = za*zbt below bf16 overflow for unclamped N(0,1) inputs.
  * A-side group sums as a 2-level bf16 pairwise tree (two 2x
    tensor_tensor adds) + one 1x tensor_reduce over the last 64: ~1.5us
    vs 2.2us flat reduce, vs 3.9us accum-TS, vs 5.4us ACT-accum.
  * Product out = (za*zbt) * (ra x rp): W = za*zbt (2x TT), scale tile
    S[p,t,c] = ra[p,t,g(c)]*rp_{g(t)}[c] built on GpSimd as 8 small
    tensor_scalar_muls (f32 rp row * f32 ra per-partition scalar -> bf16),
    final = W*S (2x TT). DVE does 2 passes at 0.52ns/elem instead of
    8 1x scalar_tensor_tensors.
  * Diagonal slots: ra-apply (wa) moved to ACT (Copy activation with
    per-partition scale), PE transpose path unchanged.
  * Stores on the GpSimd DMA ring, deferred one slot.

Per-slot engine budget (off): ACT 4.0us, DVE ~5.5us, GpSimd ~4.9us,
PE ~2.5us, DMA ~3.7us -> projected walls DVE ~91us, others below.

Per-core HBM traffic: 15*1 MiB + 2*0.5 MiB loads + 17*0.5 MiB stores
= 24.5 MiB.
"""
import sys

import numpy as np

sys.path.insert(0, "/opt/trn_rl_repo")

from contextlib import ExitStack

import concourse.bass as bass  # noqa: F401  (registers AP machinery)
import concourse.tile as tile
from concourse import bacc, masks, mybir
from concourse.bass_utils import run_bass_kernel_spmd

N = 8192          # full matrix side
B = 512           # block side
NB = N // B       # 16 blocks per side
P = 128           # SBUF partitions
T = B // P        # 4 row-subtiles per block
GRP = 256         # softmax group width
NG = B // GRP     # 2 groups per block side
TAU = 0.1
NDIAG = 2         # diagonal pairs per core (the last NDIAG slots)
NSLOTS = 17       # block-pairs per core
NOFF = NSLOTS - NDIAG
NCORES = 8
MASK = -60000.0   # pre-masked diagonal value (fp16-representable; exp->0)
EXP_BIAS = -30.0  # exp(x/TAU + EXP_BIAS): overflow headroom for za*zbt

F32 = mybir.dt.float32
F16 = mybir.dt.float16
BF16 = mybir.dt.bfloat16

AF = mybir.ActivationFunctionType
OP = mybir.AluOpType


def core_pairs() -> list[list[tuple[int, int]]]:
    """136 upper-triangle block pairs distributed 17-per-core; the 2 diagonal
    pairs of each core come last (the kernel treats those slots specially)."""
    diag = [(i, i) for i in range(NB)]
    off = [(i, j) for i in range(NB) for j in range(i + 1, NB)]
    cps: list[list[tuple[int, int]]] = [[] for _ in range(NCORES)]
    for idx, p in enumerate(off):
        cps[idx % NCORES].append(p)
    for idx, p in enumerate(diag):
        cps[idx % NCORES].append(p)
    return cps


CORE_PAIRS = core_pairs()


def build():
    """Build + compile the (single-program, 8-core SPMD) Bass kernel."""
    nc = bacc.Bacc(
        "TRN2",
        target_bir_lowering=False,
        debug=False,
        enable_asserts=True,
        num_devices=NCORES,
    )
    ab = nc.dram_tensor("ab", [NOFF, P, 2, T, B], F16, kind="ExternalInput").ap()
    ad = nc.dram_tensor("ad", [NDIAG, P, T, B], F16, kind="ExternalInput").ap()
    o = nc.dram_tensor("o", [NSLOTS, P, T, B], F16, kind="ExternalOutput").ap()

    with tile.TileContext(nc) as tc, ExitStack() as ctx:
        const_pool = ctx.enter_context(tc.tile_pool(name="const", bufs=1))
        ident = const_pool.tile([P, P], BF16)
        masks.make_identity(nc, ident[:])
        # All-ones stationary: one matmul both colsums zbt's partition groups
        # AND broadcasts the result to all 128 PSUM partitions. bf16 so the
        # matmuls run in one pass (fp32 matmul = 2 passes).
        ones_mat = const_pool.tile([P, P], BF16)
        nc.gpsimd.memset(ones_mat[:], 1.0)
        bias_sb = const_pool.tile([P, 1], F32)
        nc.gpsimd.memset(bias_sb[:], EXP_BIAS)

        ab_pool = ctx.enter_context(tc.tile_pool(name="ab_sb", bufs=5))
        ad_pool = ctx.enter_context(tc.tile_pool(name="ad_sb", bufs=2))
        za_pool = ctx.enter_context(tc.tile_pool(name="za", bufs=3))
        zb_pool = ctx.enter_context(tc.tile_pool(name="zbt", bufs=3))
        w_pool = ctx.enter_context(tc.tile_pool(name="w", bufs=4))
        s_pool = ctx.enter_context(tc.tile_pool(name="s", bufs=4))
        h_pool = ctx.enter_context(tc.tile_pool(name="h", bufs=4))
        o_pool = ctx.enter_context(tc.tile_pool(name="o_sb", bufs=4))
        st_pool = ctx.enter_context(tc.tile_pool(name="st", bufs=10))
        rp_pool = ctx.enter_context(tc.tile_pool(name="rp", bufs=4))
        dg_pool = ctx.enter_context(tc.tile_pool(name="dg", bufs=2))
        ps_pool = ctx.enter_context(tc.tile_pool(name="ps", bufs=4, space="PSUM"))

        def tree_sums(za, sa, ra):
            """sa[p, t, g] = sum_c za[p, t, g*256+c]; ra = 1/sa.
            Two 2x-mode bf16 pairwise-add stages + one small 1x reduce,
            all through one scratch tile (fewer pool alloc/release syncs)."""
            za4 = za.rearrange("p t (g c) -> p (t g) c", c=GRP)
            h = h_pool.tile([P, T * NG, GRP // 2 + GRP // 4], BF16)
            h1 = h[:, :, 0:128]
            h2 = h[:, :, 128:192]
            nc.vector.tensor_tensor(h1, za4[:, :, 0:128], za4[:, :, 128:256],
                                    op=OP.add)
            nc.vector.tensor_tensor(h2, h1[:, :, 0:64], h1[:, :, 64:128],
                                    op=OP.add)
            nc.vector.tensor_reduce(sa.rearrange("p t g -> p (t g)"), h2,
                                    axis=mybir.AxisListType.X, op=OP.add)
            nc.vector.reciprocal(ra.rearrange("p t g -> p (t g)"),
                                 sa.rearrange("p t g -> p (t g)"))

        # Diagonal slots are interleaved mid-program: their short chains give
        # ACT/DVE low-dependency filler work between full off-slot chains.
        order = [*range(0, 7), NOFF, *range(7, 12), NOFF + 1, *range(12, NOFF)]
        # Stores are deferred one slot: issued immediately, store(k) sits at
        # the GpSimd queue head waiting on slot k's full product and blocks
        # slot k+1's work behind it (head-of-line serialization).
        pending_store = None
        for k in order:
            diag_slot = k >= NOFF
            if not diag_slot:
                # --- off-diagonal pair: A and host-pre-transposed B ---
                ab_sb = ab_pool.tile([P, 2, T, B], F16)
                nc.sync.dma_start(ab_sb[:], ab[k])

                # Separate za/zbt tiles: DVE reads za while GpSimd/PE read
                # zbt -- a merged tile measured 2.4x slower DVE tensor_scalars
                # (SBUF bank contention). BT exp split so PE starts early.
                zbt_t = zb_pool.tile([P, T, B], BF16, name="zbt")
                za_t = za_pool.tile([P, T, B], BF16, name="za")
                za = za_t[:]
                zbt = zbt_t[:]
                s_ps = ps_pool.tile([P, NG, B], F32, name="p23")
                for g in range(NG):
                    ts = slice(NG * g, NG * (g + 1))
                    nc.scalar.activation(zbt_t[:, ts, :], ab_sb[:, 1, ts, :],
                                         AF.Exp, scale=1.0 / TAU,
                                         bias=bias_sb[:])
                    for u in range(NG):
                        nc.tensor.matmul(
                            s_ps[:, g, :], ones_mat[:], zbt[:, g * NG + u, :],
                            start=(u == 0), stop=(u == NG - 1),
                        )
                for h in range(NG):
                    ts = slice(NG * h, NG * (h + 1))
                    nc.scalar.activation(za_t[:, ts, :], ab_sb[:, 0, ts, :],
                                         AF.Exp, scale=1.0 / TAU,
                                         bias=bias_sb[:])
                rp_sb = rp_pool.tile([P, NG, B], F32)
                nc.vector.reciprocal_approx_fast(
                    rp_sb[:].rearrange("p g b -> p (g b)"),
                    s_ps[:].rearrange("p g b -> p (g b)"))

                # A side: group sums + ra.
                sr = st_pool.tile([P, 2, T, NG], F32, name="sr")
                sa, ra = sr[:, 0], sr[:, 1]
                tree_sums(za, sa, ra)

                # X = bhB.T in fp16, fully on GpSimd tensor_tensors (2 ops,
                # rp broadcast across the t-pair; GpSimd TT measured
                # 2.25ns/elem -- its tensor_scalar is 7x slower, avoid).
                x_sb = s_pool.tile([P, T, B], BF16)
                for h in range(NG):
                    ts = slice(NG * h, NG * (h + 1))
                    nc.gpsimd.tensor_mul(
                        x_sb[:, ts, :], zbt[:, ts, :],
                        rp_sb[:, h:h + 1, :].broadcast_to([P, NG, B]),
                    )

                # wa = za*ra via 2x-mode tensor_scalar_mul (253ns/region
                # measured), then ONE 2x-mode tensor_tensor for out = wa*X.
                wa = w_pool.tile([P, T, B], BF16)
                o_sb = o_pool.tile([P, T, B], F16)
                for t in range(T):
                    for g in range(NG):
                        cs = slice(g * GRP, (g + 1) * GRP)
                        nc.vector.tensor_scalar_mul(
                            wa[:, t, cs], za[:, t, cs], ra[:, t, g:g + 1])
                nc.vector.tensor_tensor(o_sb[:], wa[:], x_sb[:], op=OP.mult)
            else:
                # --- diagonal pair: B == A, PE bf16 transpose ---
                a_sb = ad_pool.tile([P, T, B], F16)
                nc.sync.dma_start(a_sb[:], ad[k - NOFF])
                zad = za_pool.tile([P, T, B], BF16)
                for h in range(NG):
                    ts = slice(NG * h, NG * (h + 1))
                    nc.scalar.activation(zad[:, ts, :], a_sb[:, ts, :],
                                         AF.Exp, scale=1.0 / TAU,
                                         bias=bias_sb[:])
                sr = st_pool.tile([P, 2, T, NG], F32, name="sr")
                sa, ra = sr[:, 0], sr[:, 1]
                tree_sums(zad[:], sa, ra)
                wa = w_pool.tile([P, T, B], BF16)
                for t in range(T):
                    for g in range(NG):
                        cs = slice(g * GRP, (g + 1) * GRP)
                        nc.vector.tensor_scalar_mul(
                            wa[:, t, cs], zad[:, t, cs], ra[:, t, g:g + 1])
                dg = dg_pool.tile([P, T * NG, P], BF16)
                nc.gpsimd.tensor_mul(
                    dg[:],
                    ident[:].rearrange("p (one c) -> p one c", one=1)
                    .broadcast_to([P, T * NG, P]),
                    ra[:].rearrange("p t g -> p (t g)")
                    .rearrange("p (tg one) -> p tg one", one=1)
                    .broadcast_to([P, T * NG, P]),
                )
                # Two v-waves through one 2-bank PSUM tile; wave 2 reuses the
                # banks after wave 1's products are read.
                p23 = ps_pool.tile([P, NG, B], F32, name="p23")
                o_sb = o_pool.tile([P, T, B], F16)
                for w in range(NG):
                    for hv in range(NG):
                        v = w * NG + hv
                        for u in range(T):
                            nc.tensor.matmul(
                                p23[:, hv, u * P:(u + 1) * P],
                                zad[:, u, v * P:(v + 1) * P],
                                dg[:, u * NG + (v // NG), :],
                            )
                    nc.vector.tensor_tensor(
                        o_sb[:, w * NG:(w + 1) * NG, :],
                        wa[:, w * NG:(w + 1) * NG, :], p23[:],
                        op=OP.mult)

            # One whole-block store per slot on the SWDGE (gpsimd) ring: it
            # never queues ahead of loads on the sync HWDGE ring.
            if pending_store is not None:
                eng = nc.sync if pending_store[0] % 2 == 1 else nc.gpsimd
                eng.dma_start(o[pending_store[0]], pending_store[1][:])
            pending_store = (k, o_sb)
        eng = nc.sync if pending_store[0] % 2 == 1 else nc.gpsimd
        eng.dma_start(o[pending_store[0]], pending_store[1][:])

    nc.compile()
    return nc


_NC = None


def _get_nc():
    global _NC
    if _NC is None:
        _NC = build()
    return _NC


def _to_pmajor(block: np.ndarray) -> np.ndarray:
    # (512, 512) row-major -> (128, 4, 512): row r = t*P + p lands at
    # [p, t, :], so every SBUF partition's bytes are contiguous in DRAM.
    return block.reshape(T, P, B).transpose(1, 0, 2)


def make_in_maps(sims: np.ndarray) -> list[dict[str, np.ndarray]]:
    in_maps = []
    for c in range(NCORES):
        ab_stack = np.empty((NOFF, P, 2, T, B), np.float16)
        ad_stack = np.empty((NDIAG, P, T, B), np.float16)
        for k, (i, j) in enumerate(CORE_PAIRS[c]):
            if k < NOFF:
                assert i != j
                ab_stack[k, :, 0] = _to_pmajor(
                    sims[i * B:(i + 1) * B, j * B:(j + 1) * B]).astype(
                        np.float16)
                ab_stack[k, :, 1] = _to_pmajor(
                    np.ascontiguousarray(
                        sims[j * B:(j + 1) * B, i * B:(i + 1) * B].T)).astype(
                            np.float16)
            else:
                assert i == j
                a = sims[i * B:(i + 1) * B, i * B:(i + 1) * B].copy()
                np.fill_diagonal(a, MASK)
                ad_stack[k - NOFF] = _to_pmajor(a).astype(np.float16)
        in_maps.append({"ab": ab_stack, "ad": ad_stack})
    return in_maps


def assemble(results: list[dict[str, np.ndarray]]) -> np.ndarray:
    out = np.empty((N, N), np.float32)
    for c in range(NCORES):
        o_pm = results[c]["o"]  # (NSLOTS, P, T, B) fp16, partition-major
        o_stack = o_pm.astype(np.float32).transpose(0, 2, 1, 3).reshape(
            NSLOTS, B, B)
        for k, (i, j) in enumerate(CORE_PAIRS[c]):
            out[i * B:(i + 1) * B, j * B:(j + 1) * B] = o_stack[k]
            if i != j:
                out[j * B:(j + 1) * B, i * B:(i + 1) * B] = o_stack[k].T
    return out


def run_on_hw(sims: np.ndarray, **spmd_kwargs):
    """Run the kernel on the 8 NeuronCores. Returns (out, BassKernelResults).

    The device occasionally throws a transient NRT_EXEC_UNIT_UNRECOVERABLE
    and needs ~a minute to come back, so failed runs are retried."""
    import time

    nc = _get_nc()
    in_maps = make_in_maps(sims)
    last_exc = None
    for attempt in range(3):
        if attempt:
            time.sleep(75)
        try:
            res = run_bass_kernel_spmd(
                nc, in_maps, core_ids=list(range(NCORES)), **spmd_kwargs
            )
            return assemble(res.results), res
        except Exception as exc:  # noqa: BLE001 - device flake, retry
            last_exc = exc
    raise last_exc


def kernel(similarities: np.ndarray) -> np.ndarray:
    sims = np.ascontiguousarray(similarities, dtype=np.float32)
    assert sims.shape == (N, N)
    out, _ = run_on_hw(sims)
    return out


if __name__ == "__main__":
    rng = np.random.default_rng(0)
    sims = rng.standard_normal((N, N), dtype=np.float32)
    out = kernel(similarities=sims)
    print("out", out.shape, out.dtype, float(out.max()))


# revision 13
# speedup vs baseline: 3.4280x; 1.0418x over previous
"""Trainium2 Bass kernel for nn_BestHits: out = bh * bh.T where
bh = blockwise-softmax(mask_diag(similarities) / TAU) over 256-wide column groups.

Strategy: out is symmetric (out.T = bh.T * bh = out), so only the upper
triangle of 512x512 block-pairs is computed on device. The 16x16 block grid
has 136 upper-incl-diagonal pairs = 17 per core on 8 cores (each core gets
exactly 2 diagonal + 15 off-diagonal pairs -> perfectly uniform SPMD work).
B-side blocks are staged pre-transposed by the host (layout-only, free).

v3 (measured-rate driven; v1 was 142.6us with ACT 113.6/DVE 111/DMA 104.7
walls; v2's tensor_scalar+accum experiment measured: TT/TS at 2x with
all-16-bit packed operands, accum-TS stuck at 1x + READ_ACCUMULATOR,
GpSimd TT at ~2.1ns/elem):

  * Inputs staged fp16 on the host (free): 16 MiB loads/core vs 32.
  * One merged [P, side, t-pair, B] exp per t-pair covers BOTH the A and
    BT halves in a single big ACTIVATE (2 per off slot, 1137ns/1024e rate)
    with bias=-30 folded in: exp(x/TAU - 30) rescales both softmax
    numerator and denominator consistently (out invariant) and keeps
    W = za*zbt below bf16 overflow for unclamped N(0,1) inputs.
  * A-side group sums as a 2-level bf16 pairwise tree (two 2x
    tensor_tensor adds) + one 1x tensor_reduce over the last 64: ~1.5us
    vs 2.2us flat reduce, vs 3.9us accum-TS, vs 5.4us ACT-accum.
  * Product out = (za*zbt) * (ra x rp): W = za*zbt (2x TT), scale tile
    S[p,t,c] = ra[p,t,g(c)]*rp_{g(t)}[c] built on GpSimd as 8 small
    tensor_scalar_muls (f32 rp row * f32 ra per-partition scalar -> bf16),
    final = W*S (2x TT). DVE does 2 passes at 0.52ns/elem instead of
    8 1x scalar_tensor_tensors.
  * Diagonal slots: ra-apply (wa) moved to ACT (Copy activation with
    per-partition scale), PE transpose path unchanged.
  * Stores on the GpSimd DMA ring, deferred one slot.

Per-slot engine budget (off): ACT 4.0us, DVE ~5.5us, GpSimd ~4.9us,
PE ~2.5us, DMA ~3.7us -> projected walls DVE ~91us, others below.

Per-core HBM traffic: 15*1 MiB + 2*0.5 MiB loads + 17*0.5 MiB stores
= 24.5 MiB.
"""
import sys

import numpy as np

sys.path.insert(0, "/opt/trn_rl_repo")

from contextlib import ExitStack

import concourse.bass as bass  # noqa: F401  (registers AP machinery)
import concourse.tile as tile
from concourse import bacc, masks, mybir
from concourse.bass_utils import run_bass_kernel_spmd

N = 8192          # full matrix side
B = 512           # block side
NB = N // B       # 16 blocks per side
P = 128           # SBUF partitions
T = B // P        # 4 row-subtiles per block
GRP = 256         # softmax group width
NG = B // GRP     # 2 groups per block side
TAU = 0.1
NDIAG = 2         # diagonal pairs per core (the last NDIAG slots)
NSLOTS = 17       # block-pairs per core
NOFF = NSLOTS - NDIAG
NCORES = 8
MASK = -60000.0   # pre-masked diagonal value (fp16-representable; exp->0)
EXP_BIAS = -30.0  # exp(x/TAU + EXP_BIAS): overflow headroom for za*zbt

F32 = mybir.dt.float32
F16 = mybir.dt.float16
BF16 = mybir.dt.bfloat16

AF = mybir.ActivationFunctionType
OP = mybir.AluOpType


def core_pairs() -> list[list[tuple[int, int]]]:
    """136 upper-triangle block pairs distributed 17-per-core; the 2 diagonal
    pairs of each core come last (the kernel treats those slots specially)."""
    diag = [(i, i) for i in range(NB)]
    off = [(i, j) for i in range(NB) for j in range(i + 1, NB)]
    cps: list[list[tuple[int, int]]] = [[] for _ in range(NCORES)]
    for idx, p in enumerate(off):
        cps[idx % NCORES].append(p)
    for idx, p in enumerate(diag):
        cps[idx % NCORES].append(p)
    return cps


CORE_PAIRS = core_pairs()


def build():
    """Build + compile the (single-program, 8-core SPMD) Bass kernel."""
    nc = bacc.Bacc(
        "TRN2",
        target_bir_lowering=False,
        debug=False,
        enable_asserts=True,
        num_devices=NCORES,
    )
    ab = nc.dram_tensor("ab", [NOFF, P, 2, T, B], F16, kind="ExternalInput").ap()
    ad = nc.dram_tensor("ad", [NDIAG, P, T, B], F16, kind="ExternalInput").ap()
    o = nc.dram_tensor("o", [NSLOTS, P, T, B], F16, kind="ExternalOutput").ap()

    with tile.TileContext(nc) as tc, ExitStack() as ctx:
        const_pool = ctx.enter_context(tc.tile_pool(name="const", bufs=1))
        ident = const_pool.tile([P, P], BF16)
        masks.make_identity(nc, ident[:])
        # All-ones stationary: one matmul both colsums zbt's partition groups
        # AND broadcasts the result to all 128 PSUM partitions. bf16 so the
        # matmuls run in one pass (fp32 matmul = 2 passes).
        ones_mat = const_pool.tile([P, P], BF16)
        nc.gpsimd.memset(ones_mat[:], 1.0)
        bias_sb = const_pool.tile([P, 1], F32)
        nc.gpsimd.memset(bias_sb[:], EXP_BIAS)

        ab_pool = ctx.enter_context(tc.tile_pool(name="ab_sb", bufs=5))
        ad_pool = ctx.enter_context(tc.tile_pool(name="ad_sb", bufs=2))
        za_pool = ctx.enter_context(tc.tile_pool(name="za", bufs=3))
        zb_pool = ctx.enter_context(tc.tile_pool(name="zbt", bufs=3))
        w_pool = ctx.enter_context(tc.tile_pool(name="w", bufs=4))
        s_pool = ctx.enter_context(tc.tile_pool(name="s", bufs=4))
        h_pool = ctx.enter_context(tc.tile_pool(name="h", bufs=4))
        o_pool = ctx.enter_context(tc.tile_pool(name="o_sb", bufs=4))
        st_pool = ctx.enter_context(tc.tile_pool(name="st", bufs=10))
        rp_pool = ctx.enter_context(tc.tile_pool(name="rp", bufs=4))
        rb_pool = ctx.enter_context(tc.tile_pool(name="rpb", bufs=4))
        dg_pool = ctx.enter_context(tc.tile_pool(name="dg", bufs=2))
        ps_pool = ctx.enter_context(tc.tile_pool(name="ps", bufs=4, space="PSUM"))

        def tree_sums(za, sa, ra):
            """sa[p, t, g] = sum_c za[p, t, g*256+c]; ra = 1/sa.
            Two 2x-mode bf16 pairwise-add stages + one small 1x reduce,
            all through one scratch tile (fewer pool alloc/release syncs)."""
            za4 = za.rearrange("p t (g c) -> p (t g) c", c=GRP)
            h = h_pool.tile([P, T * NG, GRP // 2 + GRP // 4], BF16)
            h1 = h[:, :, 0:128]
            h2 = h[:, :, 128:192]
            nc.vector.tensor_tensor(h1, za4[:, :, 0:128], za4[:, :, 128:256],
                                    op=OP.add)
            nc.vector.tensor_tensor(h2, h1[:, :, 0:64], h1[:, :, 64:128],
                                    op=OP.add)
            nc.vector.tensor_reduce(sa.rearrange("p t g -> p (t g)"), h2,
                                    axis=mybir.AxisListType.X, op=OP.add)
            nc.vector.reciprocal(ra.rearrange("p t g -> p (t g)"),
                                 sa.rearrange("p t g -> p (t g)"))

        # Diagonal slots are interleaved mid-program: their short chains give
        # ACT/DVE low-dependency filler work between full off-slot chains.
        order = [*range(0, 7), NOFF, *range(7, 12), NOFF + 1, *range(12, NOFF)]
        # Stores are deferred one slot: issued immediately, store(k) sits at
        # the GpSimd queue head waiting on slot k's full product and blocks
        # slot k+1's work behind it (head-of-line serialization).
        pending_store = None
        for k in order:
            diag_slot = k >= NOFF
            if not diag_slot:
                # --- off-diagonal pair: A and host-pre-transposed B ---
                ab_sb = ab_pool.tile([P, 2, T, B], F16)
                nc.sync.dma_start(ab_sb[:], ab[k])

                # Separate za/zbt tiles: DVE reads za while GpSimd/PE read
                # zbt -- a merged tile measured 2.4x slower DVE tensor_scalars
                # (SBUF bank contention). BT exp split so PE starts early.
                zbt_t = zb_pool.tile([P, T, B], BF16, name="zbt")
                za_t = za_pool.tile([P, T, B], BF16, name="za")
                za = za_t[:]
                zbt = zbt_t[:]
                s_ps = ps_pool.tile([P, NG, B], F32, name="p23")
                for g in range(NG):
                    ts = slice(NG * g, NG * (g + 1))
                    nc.scalar.activation(zbt_t[:, ts, :], ab_sb[:, 1, ts, :],
                                         AF.Exp, scale=1.0 / TAU,
                                         bias=bias_sb[:])
                    for u in range(NG):
                        nc.tensor.matmul(
                            s_ps[:, g, :], ones_mat[:], zbt[:, g * NG + u, :],
                            start=(u == 0), stop=(u == NG - 1),
                        )
                for h in range(NG):
                    ts = slice(NG * h, NG * (h + 1))
                    nc.scalar.activation(za_t[:, ts, :], ab_sb[:, 0, ts, :],
                                         AF.Exp, scale=1.0 / TAU,
                                         bias=bias_sb[:])
                rp_sb = rp_pool.tile([P, NG, B], F32)
                nc.vector.reciprocal_approx_fast(
                    rp_sb[:].rearrange("p g b -> p (g b)"),
                    s_ps[:].rearrange("p g b -> p (g b)"))

                # A side: group sums + ra.
                sr = st_pool.tile([P, 2, T, NG], F32, name="sr")
                sa, ra = sr[:, 0], sr[:, 1]
                tree_sums(za, sa, ra)

                # rp -> bf16 on ACT (headroom engine), then X = zbt*rp as a
                # single 2x-mode DVE tensor_tensor. GpSimd compute is banned:
                # measured, its SBUF traffic steals ~80% of its runtime from
                # concurrent DVE throughput (wa ops 274ns -> 2150ns).
                rp_bf = rb_pool.tile([P, NG, B], BF16)
                nc.scalar.copy(rp_bf[:], rp_sb[:])
                x_sb = s_pool.tile([P, T, B], BF16)
                nc.vector.tensor_tensor(
                    x_sb[:].rearrange("p (h u) b -> p h u b", h=NG),
                    zbt.rearrange("p (h u) b -> p h u b", h=NG),
                    rp_bf[:].rearrange("p g (one b) -> p g one b", one=1)
                    .broadcast_to([P, NG, NG, B]),
                    op=OP.mult)

                # wa = za*ra via 2x-mode tensor_scalar_mul (253ns/region
                # measured), then ONE 2x-mode tensor_tensor for out = wa*X.
                wa = w_pool.tile([P, T, B], BF16)
                o_sb = o_pool.tile([P, T, B], F16)
                for t in range(T):
                    for g in range(NG):
                        cs = slice(g * GRP, (g + 1) * GRP)
                        nc.vector.tensor_scalar_mul(
                            wa[:, t, cs], za[:, t, cs], ra[:, t, g:g + 1])
                nc.vector.tensor_tensor(o_sb[:], wa[:], x_sb[:], op=OP.mult)
            else:
                # --- diagonal pair: B == A, PE bf16 transpose ---
                a_sb = ad_pool.tile([P, T, B], F16)
                nc.sync.dma_start(a_sb[:], ad[k - NOFF])
                zad = za_pool.tile([P, T, B], BF16)
                for h in range(NG):
                    ts = slice(NG * h, NG * (h + 1))
                    nc.scalar.activation(zad[:, ts, :], a_sb[:, ts, :],
                                         AF.Exp, scale=1.0 / TAU,
                                         bias=bias_sb[:])
                sr = st_pool.tile([P, 2, T, NG], F32, name="sr")
                sa, ra = sr[:, 0], sr[:, 1]
                tree_sums(zad[:], sa, ra)
                wa = w_pool.tile([P, T, B], BF16)
                for t in range(T):
                    for g in range(NG):
                        cs = slice(g * GRP, (g + 1) * GRP)
                        nc.vector.tensor_scalar_mul(
                            wa[:, t, cs], zad[:, t, cs], ra[:, t, g:g + 1])
                dg = dg_pool.tile([P, T * NG, P], BF16)
                for t in range(T):
                    for g in range(NG):
                        nc.vector.tensor_scalar_mul(
                            dg[:, t * NG + g, :], ident[:],
                            ra[:, t, g:g + 1])
                # Two v-waves through one 2-bank PSUM tile; wave 2 reuses the
                # banks after wave 1's products are read.
                p23 = ps_pool.tile([P, NG, B], F32, name="p23")
                o_sb = o_pool.tile([P, T, B], F16)
                for w in range(NG):
                    for hv in range(NG):
                        v = w * NG + hv
                        for u in range(T):
                            nc.tensor.matmul(
                                p23[:, hv, u * P:(u + 1) * P],
                                zad[:, u, v * P:(v + 1) * P],
                                dg[:, u * NG + (v // NG), :],
                            )
                    nc.vector.tensor_tensor(
                        o_sb[:, w * NG:(w + 1) * NG, :],
                        wa[:, w * NG:(w + 1) * NG, :], p23[:],
                        op=OP.mult)

            # One whole-block store per slot on the SWDGE (gpsimd) ring: it
            # never queues ahead of loads on the sync HWDGE ring.
            if pending_store is not None:
                nc.gpsimd.dma_start(o[pending_store[0]], pending_store[1][:])
            pending_store = (k, o_sb)
        nc.gpsimd.dma_start(o[pending_store[0]], pending_store[1][:])

    nc.compile()
    return nc


_NC = None


def _get_nc():
    global _NC
    if _NC is None:
        _NC = build()
    return _NC


def _to_pmajor(block: np.ndarray) -> np.ndarray:
    # (512, 512) row-major -> (128, 4, 512): row r = t*P + p lands at
    # [p, t, :], so every SBUF partition's bytes are contiguous in DRAM.
    return block.reshape(T, P, B).transpose(1, 0, 2)


def make_in_maps(sims: np.ndarray) -> list[dict[str, np.ndarray]]:
    in_maps = []
    for c in range(NCORES):
        ab_stack = np.empty((NOFF, P, 2, T, B), np.float16)
        ad_stack = np.empty((NDIAG, P, T, B), np.float16)
        for k, (i, j) in enumerate(CORE_PAIRS[c]):
            if k < NOFF:
                assert i != j
                ab_stack[k, :, 0] = _to_pmajor(
                    sims[i * B:(i + 1) * B, j * B:(j + 1) * B]).astype(
                        np.float16)
                ab_stack[k, :, 1] = _to_pmajor(
                    np.ascontiguousarray(
                        sims[j * B:(j + 1) * B, i * B:(i + 1) * B].T)).astype(
                            np.float16)
            else:
                assert i == j
                a = sims[i * B:(i + 1) * B, i * B:(i + 1) * B].copy()
                np.fill_diagonal(a, MASK)
                ad_stack[k - NOFF] = _to_pmajor(a).astype(np.float16)
        in_maps.append({"ab": ab_stack, "ad": ad_stack})
    return in_maps


def assemble(results: list[dict[str, np.ndarray]]) -> np.ndarray:
    out = np.empty((N, N), np.float32)
    for c in range(NCORES):
        o_pm = results[c]["o"]  # (NSLOTS, P, T, B) fp16, partition-major
        o_stack = o_pm.astype(np.float32).transpose(0, 2, 1, 3).reshape(
            NSLOTS, B, B)
        for k, (i, j) in enumerate(CORE_PAIRS[c]):
            out[i * B:(i + 1) * B, j * B:(j + 1) * B] = o_stack[k]
            if i != j:
                out[j * B:(j + 1) * B, i * B:(i + 1) * B] = o_stack[k].T
    return out


def run_on_hw(sims: np.ndarray, **spmd_kwargs):
    """Run the kernel on the 8 NeuronCores. Returns (out, BassKernelResults).

    The device occasionally throws a transient NRT_EXEC_UNIT_UNRECOVERABLE
    and needs ~a minute to come back, so failed runs are retried."""
    import time

    nc = _get_nc()
    in_maps = make_in_maps(sims)
    last_exc = None
    for attempt in range(3):
        if attempt:
            time.sleep(75)
        try:
            res = run_bass_kernel_spmd(
                nc, in_maps, core_ids=list(range(NCORES)), **spmd_kwargs
            )
            return assemble(res.results), res
        except Exception as exc:  # noqa: BLE001 - device flake, retry
            last_exc = exc
    raise last_exc


def kernel(similarities: np.ndarray) -> np.ndarray:
    sims = np.ascontiguousarray(similarities, dtype=np.float32)
    assert sims.shape == (N, N)
    out, _ = run_on_hw(sims)
    return out


if __name__ == "__main__":
    rng = np.random.default_rng(0)
    sims = rng.standard_normal((N, N), dtype=np.float32)
    out = kernel(similarities=sims)
    print("out", out.shape, out.dtype, float(out.max()))


# revision 16
# speedup vs baseline: 3.7947x; 1.1070x over previous
"""Trainium2 Bass kernel for nn_BestHits: out = bh * bh.T where
bh = blockwise-softmax(mask_diag(similarities) / TAU) over 256-wide column groups.

Strategy: out is symmetric (out.T = bh.T * bh = out), so only the upper
triangle of 512x512 block-pairs is computed on device. The 16x16 block grid
has 136 upper-incl-diagonal pairs = 17 per core on 8 cores (each core gets
exactly 2 diagonal + 15 off-diagonal pairs -> perfectly uniform SPMD work).
B-side blocks are staged pre-transposed by the host (layout-only, free).

v3 (measured-rate driven; v1 was 142.6us with ACT 113.6/DVE 111/DMA 104.7
walls; v2's tensor_scalar+accum experiment measured: TT/TS at 2x with
all-16-bit packed operands, accum-TS stuck at 1x + READ_ACCUMULATOR,
GpSimd TT at ~2.1ns/elem):

  * Inputs staged fp16 on the host (free): 16 MiB loads/core vs 32.
  * One merged [P, side, t-pair, B] exp per t-pair covers BOTH the A and
    BT halves in a single big ACTIVATE (2 per off slot, 1137ns/1024e rate)
    with bias=-30 folded in: exp(x/TAU - 30) rescales both softmax
    numerator and denominator consistently (out invariant) and keeps
    W = za*zbt below bf16 overflow for unclamped N(0,1) inputs.
  * A-side group sums as a 2-level bf16 pairwise tree (two 2x
    tensor_tensor adds) + one 1x tensor_reduce over the last 64: ~1.5us
    vs 2.2us flat reduce, vs 3.9us accum-TS, vs 5.4us ACT-accum.
  * Product out = (za*zbt) * (ra x rp): W = za*zbt (2x TT), scale tile
    S[p,t,c] = ra[p,t,g(c)]*rp_{g(t)}[c] built on GpSimd as 8 small
    tensor_scalar_muls (f32 rp row * f32 ra per-partition scalar -> bf16),
    final = W*S (2x TT). DVE does 2 passes at 0.52ns/elem instead of
    8 1x scalar_tensor_tensors.
  * Diagonal slots: ra-apply (wa) moved to ACT (Copy activation with
    per-partition scale), PE transpose path unchanged.
  * Stores on the GpSimd DMA ring, deferred one slot.

Per-slot engine budget (off): ACT 4.0us, DVE ~5.5us, GpSimd ~4.9us,
PE ~2.5us, DMA ~3.7us -> projected walls DVE ~91us, others below.

Per-core HBM traffic: 15*1 MiB + 2*0.5 MiB loads + 17*0.5 MiB stores
= 24.5 MiB.
"""
import sys

import numpy as np

sys.path.insert(0, "/opt/trn_rl_repo")

from contextlib import ExitStack

import concourse.bass as bass  # noqa: F401  (registers AP machinery)
import concourse.tile as tile
from concourse import bacc, masks, mybir
from concourse.bass_utils import run_bass_kernel_spmd

N = 8192          # full matrix side
B = 512           # block side
NB = N // B       # 16 blocks per side
P = 128           # SBUF partitions
T = B // P        # 4 row-subtiles per block
GRP = 256         # softmax group width
NG = B // GRP     # 2 groups per block side
TAU = 0.1
NDIAG = 2         # diagonal pairs per core (the last NDIAG slots)
NSLOTS = 17       # block-pairs per core
NOFF = NSLOTS - NDIAG
NCORES = 8
MASK = -60000.0   # pre-masked diagonal value (fp16-representable; exp->0)
EXP_BIAS = -30.0  # exp(x/TAU + EXP_BIAS): overflow headroom for za*zbt

F32 = mybir.dt.float32
F16 = mybir.dt.float16
BF16 = mybir.dt.bfloat16

AF = mybir.ActivationFunctionType
OP = mybir.AluOpType


def core_pairs() -> list[list[tuple[int, int]]]:
    """136 upper-triangle block pairs distributed 17-per-core; the 2 diagonal
    pairs of each core come last (the kernel treats those slots specially)."""
    diag = [(i, i) for i in range(NB)]
    off = [(i, j) for i in range(NB) for j in range(i + 1, NB)]
    cps: list[list[tuple[int, int]]] = [[] for _ in range(NCORES)]
    for idx, p in enumerate(off):
        cps[idx % NCORES].append(p)
    for idx, p in enumerate(diag):
        cps[idx % NCORES].append(p)
    return cps


CORE_PAIRS = core_pairs()


def build():
    """Build + compile the (single-program, 8-core SPMD) Bass kernel."""
    nc = bacc.Bacc(
        "TRN2",
        target_bir_lowering=False,
        debug=False,
        enable_asserts=True,
        num_devices=NCORES,
    )
    ab = nc.dram_tensor("ab", [NSLOTS, P, 2, T, B], F16,
                        kind="ExternalInput").ap()
    o = nc.dram_tensor("o", [NSLOTS, P, T, B], F16, kind="ExternalOutput").ap()

    with tile.TileContext(nc) as tc, ExitStack() as ctx:
        const_pool = ctx.enter_context(tc.tile_pool(name="const", bufs=1))
        # All-ones stationary: one matmul both colsums zbt's partition groups
        # AND broadcasts the result to all 128 PSUM partitions. bf16 so the
        # matmuls run in one pass (fp32 matmul = 2 passes).
        ones_mat = const_pool.tile([P, P], BF16)
        nc.gpsimd.memset(ones_mat[:], 1.0)
        bias_sb = const_pool.tile([P, 1], F32)
        nc.gpsimd.memset(bias_sb[:], EXP_BIAS)

        ab_pool = ctx.enter_context(tc.tile_pool(name="ab_sb", bufs=5))
        za_pool = ctx.enter_context(tc.tile_pool(name="za", bufs=3))
        zb_pool = ctx.enter_context(tc.tile_pool(name="zbt", bufs=3))
        w_pool = ctx.enter_context(tc.tile_pool(name="w", bufs=4))
        s_pool = ctx.enter_context(tc.tile_pool(name="s", bufs=4))
        h_pool = ctx.enter_context(tc.tile_pool(name="h", bufs=4))
        o_pool = ctx.enter_context(tc.tile_pool(name="o_sb", bufs=4))
        st_pool = ctx.enter_context(tc.tile_pool(name="st", bufs=10))
        rp_pool = ctx.enter_context(tc.tile_pool(name="rp", bufs=4))
        rb_pool = ctx.enter_context(tc.tile_pool(name="rpb", bufs=4))
        ps_pool = ctx.enter_context(tc.tile_pool(name="ps", bufs=4, space="PSUM"))

        def tree_sums(za, sa, ra):
            """sa[p, t, g] = sum_c za[p, t, g*256+c]; ra = 1/sa.
            Two 2x-mode bf16 pairwise-add stages + one small 1x reduce,
            all through one scratch tile (fewer pool alloc/release syncs)."""
            za4 = za.rearrange("p t (g c) -> p (t g) c", c=GRP)
            h = h_pool.tile([P, T * NG, GRP // 2 + GRP // 4], BF16)
            h1 = h[:, :, 0:128]
            h2 = h[:, :, 128:192]
            nc.vector.tensor_tensor(h1, za4[:, :, 0:128], za4[:, :, 128:256],
                                    op=OP.add)
            nc.vector.tensor_tensor(h2, h1[:, :, 0:64], h1[:, :, 64:128],
                                    op=OP.add)
            nc.vector.tensor_reduce(sa.rearrange("p t g -> p (t g)"), h2,
                                    axis=mybir.AxisListType.X, op=OP.add)
            nc.vector.reciprocal(ra.rearrange("p t g -> p (t g)"),
                                 sa.rearrange("p t g -> p (t g)"))

        # Diagonal slots are interleaved mid-program: their short chains give
        # ACT/DVE low-dependency filler work between full off-slot chains.
        order = [*range(0, 7), NOFF, *range(7, 12), NOFF + 1, *range(12, NOFF)]
        # Stores are deferred one slot: issued immediately, store(k) sits at
        # the GpSimd queue head waiting on slot k's full product and blocks
        # slot k+1's work behind it (head-of-line serialization).
        pending_store = None
        for k in order:
            if True:
                # --- uniform slot: A and host-pre-transposed B (diagonal
                # pairs are staged the same way: masked A + masked A.T) ---
                ab_sb = ab_pool.tile([P, 2, T, B], F16)
                nc.sync.dma_start(ab_sb[:], ab[k])

                # Separate za/zbt tiles: DVE reads za while GpSimd/PE read
                # zbt -- a merged tile measured 2.4x slower DVE tensor_scalars
                # (SBUF bank contention). BT exp split so PE starts early.
                zbt_t = zb_pool.tile([P, T, B], BF16, name="zbt")
                za_t = za_pool.tile([P, T, B], BF16, name="za")
                za = za_t[:]
                zbt = zbt_t[:]
                s_ps = ps_pool.tile([P, NG, B], F32, name="p23")
                for g in range(NG):
                    ts = slice(NG * g, NG * (g + 1))
                    nc.scalar.activation(zbt_t[:, ts, :], ab_sb[:, 1, ts, :],
                                         AF.Exp, scale=1.0 / TAU,
                                         bias=bias_sb[:])
                    for u in range(NG):
                        nc.tensor.matmul(
                            s_ps[:, g, :], ones_mat[:], zbt[:, g * NG + u, :],
                            start=(u == 0), stop=(u == NG - 1),
                        )
                for h in range(NG):
                    ts = slice(NG * h, NG * (h + 1))
                    nc.scalar.activation(za_t[:, ts, :], ab_sb[:, 0, ts, :],
                                         AF.Exp, scale=1.0 / TAU,
                                         bias=bias_sb[:])
                rp_sb = rp_pool.tile([P, NG, B], F32)
                nc.vector.reciprocal_approx_fast(
                    rp_sb[:].rearrange("p g b -> p (g b)"),
                    s_ps[:].rearrange("p g b -> p (g b)"))

                # A side: group sums + ra.
                sr = st_pool.tile([P, 2, T, NG], F32, name="sr")
                sa, ra = sr[:, 0], sr[:, 1]
                tree_sums(za, sa, ra)

                # rp -> bf16 on ACT (headroom engine), then X = zbt*rp as a
                # single 2x-mode DVE tensor_tensor. GpSimd compute is banned:
                # measured, its SBUF traffic steals ~80% of its runtime from
                # concurrent DVE throughput (wa ops 274ns -> 2150ns).
                rp_bf = rb_pool.tile([P, NG, B], BF16)
                nc.scalar.copy(rp_bf[:], rp_sb[:])
                x_sb = s_pool.tile([P, T, B], BF16)
                nc.vector.tensor_tensor(
                    x_sb[:].rearrange("p (h u) b -> p h u b", h=NG),
                    zbt.rearrange("p (h u) b -> p h u b", h=NG),
                    rp_bf[:].rearrange("p g (one b) -> p g one b", one=1)
                    .broadcast_to([P, NG, NG, B]),
                    op=OP.mult)

                # wa = za*ra via 2x-mode tensor_scalar_mul (253ns/region
                # measured), then ONE 2x-mode tensor_tensor for out = wa*X.
                wa = w_pool.tile([P, T, B], BF16)
                o_sb = o_pool.tile([P, T, B], F16)
                nc.vector.tensor_tensor(
                    wa[:].rearrange("p t (g c) -> p (t g) c", c=GRP),
                    za.rearrange("p t (g c) -> p (t g) c", c=GRP),
                    ra.rearrange("p t (g one) -> p (t g) one", one=1)
                    .broadcast_to([P, T * NG, GRP]),
                    op=OP.mult)
                nc.vector.tensor_tensor(o_sb[:], wa[:], x_sb[:], op=OP.mult)
            # One whole-block store per slot on the SWDGE (gpsimd) ring: it
            # never queues ahead of loads on the sync HWDGE ring.
            if pending_store is not None:
                nc.gpsimd.dma_start(o[pending_store[0]], pending_store[1][:])
            pending_store = (k, o_sb)
        nc.gpsimd.dma_start(o[pending_store[0]], pending_store[1][:])

    nc.compile()
    return nc


_NC = None


def _get_nc():
    global _NC
    if _NC is None:
        _NC = build()
    return _NC


def _to_pmajor(block: np.ndarray) -> np.ndarray:
    # (512, 512) row-major -> (128, 4, 512): row r = t*P + p lands at
    # [p, t, :], so every SBUF partition's bytes are contiguous in DRAM.
    return block.reshape(T, P, B).transpose(1, 0, 2)


def make_in_maps(sims: np.ndarray) -> list[dict[str, np.ndarray]]:
    in_maps = []
    for c in range(NCORES):
        ab_stack = np.empty((NSLOTS, P, 2, T, B), np.float16)
        for k, (i, j) in enumerate(CORE_PAIRS[c]):
            if i != j:
                a = sims[i * B:(i + 1) * B, j * B:(j + 1) * B]
                bt = sims[j * B:(j + 1) * B, i * B:(i + 1) * B].T
            else:
                a = sims[i * B:(i + 1) * B, i * B:(i + 1) * B].copy()
                np.fill_diagonal(a, MASK)
                bt = a.T
            ab_stack[k, :, 0] = _to_pmajor(a).astype(np.float16)
            ab_stack[k, :, 1] = _to_pmajor(
                np.ascontiguousarray(bt)).astype(np.float16)
        in_maps.append({"ab": ab_stack})
    return in_maps


def assemble(results: list[dict[str, np.ndarray]]) -> np.ndarray:
    out = np.empty((N, N), np.float32)
    for c in range(NCORES):
        o_pm = results[c]["o"]  # (NSLOTS, P, T, B) fp16, partition-major
        o_stack = o_pm.astype(np.float32).transpose(0, 2, 1, 3).reshape(
            NSLOTS, B, B)
        for k, (i, j) in enumerate(CORE_PAIRS[c]):
            out[i * B:(i + 1) * B, j * B:(j + 1) * B] = o_stack[k]
            if i != j:
                out[j * B:(j + 1) * B, i * B:(i + 1) * B] = o_stack[k].T
    return out


def run_on_hw(sims: np.ndarray, **spmd_kwargs):
    """Run the kernel on the 8 NeuronCores. Returns (out, BassKernelResults).

    The device occasionally throws a transient NRT_EXEC_UNIT_UNRECOVERABLE
    and needs ~a minute to come back, so failed runs are retried."""
    import time

    nc = _get_nc()
    in_maps = make_in_maps(sims)
    last_exc = None
    for attempt in range(3):
        if attempt:
            time.sleep(75)
        try:
            res = run_bass_kernel_spmd(
                nc, in_maps, core_ids=list(range(NCORES)), **spmd_kwargs
            )
            return assemble(res.results), res
        except Exception as exc:  # noqa: BLE001 - device flake, retry
            last_exc = exc
    raise last_exc


def kernel(similarities: np.ndarray) -> np.ndarray:
    sims = np.ascontiguousarray(similarities, dtype=np.float32)
    assert sims.shape == (N, N)
    out, _ = run_on_hw(sims)
    return out


if __name__ == "__main__":
    rng = np.random.default_rng(0)
    sims = rng.standard_normal((N, N), dtype=np.float32)
    out = kernel(similarities=sims)
    print("out", out.shape, out.dtype, float(out.max()))


# revision 17
# speedup vs baseline: 3.8080x; 1.0035x over previous
"""Trainium2 Bass kernel for nn_BestHits: out = bh * bh.T where
bh = blockwise-softmax(mask_diag(similarities) / TAU) over 256-wide column groups.

Strategy: out is symmetric (out.T = bh.T * bh = out), so only the upper
triangle of 512x512 block-pairs is computed on device. The 16x16 block grid
has 136 upper-incl-diagonal pairs = 17 per core on 8 cores (each core gets
exactly 2 diagonal + 15 off-diagonal pairs -> perfectly uniform SPMD work).
B-side blocks are staged pre-transposed by the host (layout-only, free).

v3 (measured-rate driven; v1 was 142.6us with ACT 113.6/DVE 111/DMA 104.7
walls; v2's tensor_scalar+accum experiment measured: TT/TS at 2x with
all-16-bit packed operands, accum-TS stuck at 1x + READ_ACCUMULATOR,
GpSimd TT at ~2.1ns/elem):

  * Inputs staged fp16 on the host (free): 16 MiB loads/core vs 32.
  * One merged [P, side, t-pair, B] exp per t-pair covers BOTH the A and
    BT halves in a single big ACTIVATE (2 per off slot, 1137ns/1024e rate)
    with bias=-30 folded in: exp(x/TAU - 30) rescales both softmax
    numerator and denominator consistently (out invariant) and keeps
    W = za*zbt below bf16 overflow for unclamped N(0,1) inputs.
  * A-side group sums as a 2-level bf16 pairwise tree (two 2x
    tensor_tensor adds) + one 1x tensor_reduce over the last 64: ~1.5us
    vs 2.2us flat reduce, vs 3.9us accum-TS, vs 5.4us ACT-accum.
  * Product out = (za*zbt) * (ra x rp): W = za*zbt (2x TT), scale tile
    S[p,t,c] = ra[p,t,g(c)]*rp_{g(t)}[c] built on GpSimd as 8 small
    tensor_scalar_muls (f32 rp row * f32 ra per-partition scalar -> bf16),
    final = W*S (2x TT). DVE does 2 passes at 0.52ns/elem instead of
    8 1x scalar_tensor_tensors.
  * Diagonal slots: ra-apply (wa) moved to ACT (Copy activation with
    per-partition scale), PE transpose path unchanged.
  * Stores on the GpSimd DMA ring, deferred one slot.

Per-slot engine budget (off): ACT 4.0us, DVE ~5.5us, GpSimd ~4.9us,
PE ~2.5us, DMA ~3.7us -> projected walls DVE ~91us, others below.

Per-core HBM traffic: 15*1 MiB + 2*0.5 MiB loads + 17*0.5 MiB stores
= 24.5 MiB.
"""
import sys

import numpy as np

sys.path.insert(0, "/opt/trn_rl_repo")

from contextlib import ExitStack

import concourse.bass as bass  # noqa: F401  (registers AP machinery)
import concourse.tile as tile
from concourse import bacc, masks, mybir
from concourse.bass_utils import run_bass_kernel_spmd

N = 8192          # full matrix side
B = 512           # block side
NB = N // B       # 16 blocks per side
P = 128           # SBUF partitions
T = B // P        # 4 row-subtiles per block
GRP = 256         # softmax group width
NG = B // GRP     # 2 groups per block side
TAU = 0.1
NDIAG = 2         # diagonal pairs per core (the last NDIAG slots)
NSLOTS = 17       # block-pairs per core
NOFF = NSLOTS - NDIAG
NCORES = 8
MASK = -60000.0   # pre-masked diagonal value (fp16-representable; exp->0)
EXP_BIAS = -30.0  # exp(x/TAU + EXP_BIAS): overflow headroom for za*zbt

F32 = mybir.dt.float32
F16 = mybir.dt.float16
BF16 = mybir.dt.bfloat16

AF = mybir.ActivationFunctionType
OP = mybir.AluOpType


def core_pairs() -> list[list[tuple[int, int]]]:
    """136 upper-triangle block pairs distributed 17-per-core; the 2 diagonal
    pairs of each core come last (the kernel treats those slots specially)."""
    diag = [(i, i) for i in range(NB)]
    off = [(i, j) for i in range(NB) for j in range(i + 1, NB)]
    cps: list[list[tuple[int, int]]] = [[] for _ in range(NCORES)]
    for idx, p in enumerate(off):
        cps[idx % NCORES].append(p)
    for idx, p in enumerate(diag):
        cps[idx % NCORES].append(p)
    return cps


CORE_PAIRS = core_pairs()


def build():
    """Build + compile the (single-program, 8-core SPMD) Bass kernel."""
    nc = bacc.Bacc(
        "TRN2",
        target_bir_lowering=False,
        debug=False,
        enable_asserts=True,
        num_devices=NCORES,
    )
    ab = nc.dram_tensor("ab", [NSLOTS, P, 2, T, B], F16,
                        kind="ExternalInput").ap()
    o = nc.dram_tensor("o", [NSLOTS, P, T, B], F16, kind="ExternalOutput").ap()

    with tile.TileContext(nc) as tc, ExitStack() as ctx:
        const_pool = ctx.enter_context(tc.tile_pool(name="const", bufs=1))
        # All-ones stationary: one matmul both colsums zbt's partition groups
        # AND broadcasts the result to all 128 PSUM partitions. bf16 so the
        # matmuls run in one pass (fp32 matmul = 2 passes).
        ones_mat = const_pool.tile([P, P], BF16)
        nc.gpsimd.memset(ones_mat[:], 1.0)
        bias_sb = const_pool.tile([P, 1], F32)
        nc.gpsimd.memset(bias_sb[:], EXP_BIAS)

        ab_pool = ctx.enter_context(tc.tile_pool(name="ab_sb", bufs=5))
        za_pool = ctx.enter_context(tc.tile_pool(name="za", bufs=3))
        zb_pool = ctx.enter_context(tc.tile_pool(name="zbt", bufs=3))
        w_pool = ctx.enter_context(tc.tile_pool(name="w", bufs=4))
        s_pool = ctx.enter_context(tc.tile_pool(name="s", bufs=4))
        h_pool = ctx.enter_context(tc.tile_pool(name="h", bufs=4))
        o_pool = ctx.enter_context(tc.tile_pool(name="o_sb", bufs=4))
        st_pool = ctx.enter_context(tc.tile_pool(name="st", bufs=10))
        rp_pool = ctx.enter_context(tc.tile_pool(name="rp", bufs=4))
        rb_pool = ctx.enter_context(tc.tile_pool(name="rpb", bufs=4))
        ps_pool = ctx.enter_context(tc.tile_pool(name="ps", bufs=4, space="PSUM"))

        def tree_sums(za, sa, ra):
            """sa[p, t, g] = sum_c za[p, t, g*256+c]; ra = 1/sa.
            Two 2x-mode bf16 pairwise-add stages + one small 1x reduce,
            all through one scratch tile (fewer pool alloc/release syncs)."""
            za4 = za.rearrange("p t (g c) -> p (t g) c", c=GRP)
            h = h_pool.tile([P, T * NG, GRP // 2 + GRP // 4], BF16)
            h1 = h[:, :, 0:128]
            h2 = h[:, :, 128:192]
            nc.vector.tensor_tensor(h1, za4[:, :, 0:128], za4[:, :, 128:256],
                                    op=OP.add)
            nc.vector.tensor_tensor(h2, h1[:, :, 0:64], h1[:, :, 64:128],
                                    op=OP.add)
            nc.vector.tensor_reduce(sa.rearrange("p t g -> p (t g)"), h2,
                                    axis=mybir.AxisListType.X, op=OP.add)
            nc.vector.reciprocal(ra.rearrange("p t g -> p (t g)"),
                                 sa.rearrange("p t g -> p (t g)"))

        # Diagonal slots are interleaved mid-program: their short chains give
        # ACT/DVE low-dependency filler work between full off-slot chains.
        order = [*range(0, 7), NOFF, *range(7, 12), NOFF + 1, *range(12, NOFF)]
        # Stores are deferred one slot: issued immediately, store(k) sits at
        # the GpSimd queue head waiting on slot k's full product and blocks
        # slot k+1's work behind it (head-of-line serialization).
        pending_store = None
        for k in order:
            if True:
                # --- uniform slot: A and host-pre-transposed B (diagonal
                # pairs are staged the same way: masked A + masked A.T) ---
                ab_sb = ab_pool.tile([P, 2, T, B], F16)
                nc.sync.dma_start(ab_sb[:, 0], ab[k, :, 0])
                nc.sync.dma_start(ab_sb[:, 1], ab[k, :, 1])

                # Separate za/zbt tiles: DVE reads za while GpSimd/PE read
                # zbt -- a merged tile measured 2.4x slower DVE tensor_scalars
                # (SBUF bank contention). BT exp split so PE starts early.
                zbt_t = zb_pool.tile([P, T, B], BF16, name="zbt")
                za_t = za_pool.tile([P, T, B], BF16, name="za")
                za = za_t[:]
                zbt = zbt_t[:]
                s_ps = ps_pool.tile([P, NG, B], F32, name="p23")
                for g in range(NG):
                    ts = slice(NG * g, NG * (g + 1))
                    nc.scalar.activation(zbt_t[:, ts, :], ab_sb[:, 1, ts, :],
                                         AF.Exp, scale=1.0 / TAU,
                                         bias=bias_sb[:])
                    for u in range(NG):
                        nc.tensor.matmul(
                            s_ps[:, g, :], ones_mat[:], zbt[:, g * NG + u, :],
                            start=(u == 0), stop=(u == NG - 1),
                        )
                nc.scalar.activation(za_t[:], ab_sb[:, 0], AF.Exp,
                                     scale=1.0 / TAU, bias=bias_sb[:])
                rp_sb = rp_pool.tile([P, NG, B], F32)
                nc.vector.reciprocal_approx_fast(
                    rp_sb[:].rearrange("p g b -> p (g b)"),
                    s_ps[:].rearrange("p g b -> p (g b)"))

                # A side: group sums + ra.
                sr = st_pool.tile([P, 2, T, NG], F32, name="sr")
                sa, ra = sr[:, 0], sr[:, 1]
                tree_sums(za, sa, ra)

                # rp -> bf16 on ACT (headroom engine), then X = zbt*rp as a
                # single 2x-mode DVE tensor_tensor. GpSimd compute is banned:
                # measured, its SBUF traffic steals ~80% of its runtime from
                # concurrent DVE throughput (wa ops 274ns -> 2150ns).
                rp_bf = rb_pool.tile([P, NG, B], BF16)
                nc.scalar.copy(rp_bf[:], rp_sb[:])
                x_sb = s_pool.tile([P, T, B], BF16)
                nc.vector.tensor_tensor(
                    x_sb[:].rearrange("p (h u) b -> p h u b", h=NG),
                    zbt.rearrange("p (h u) b -> p h u b", h=NG),
                    rp_bf[:].rearrange("p g (one b) -> p g one b", one=1)
                    .broadcast_to([P, NG, NG, B]),
                    op=OP.mult)

                # wa = za*ra via 2x-mode tensor_scalar_mul (253ns/region
                # measured), then ONE 2x-mode tensor_tensor for out = wa*X.
                wa = w_pool.tile([P, T, B], BF16)
                o_sb = o_pool.tile([P, T, B], F16)
                nc.vector.tensor_tensor(
                    wa[:].rearrange("p t (g c) -> p (t g) c", c=GRP),
                    za.rearrange("p t (g c) -> p (t g) c", c=GRP),
                    ra.rearrange("p t (g one) -> p (t g) one", one=1)
                    .broadcast_to([P, T * NG, GRP]),
                    op=OP.mult)
                nc.vector.tensor_tensor(o_sb[:], wa[:], x_sb[:], op=OP.mult)
            # One whole-block store per slot on the SWDGE (gpsimd) ring: it
            # never queues ahead of loads on the sync HWDGE ring.
            if pending_store is not None:
                nc.gpsimd.dma_start(o[pending_store[0]], pending_store[1][:])
            pending_store = (k, o_sb)
        nc.gpsimd.dma_start(o[pending_store[0]], pending_store[1][:])

    nc.compile()
    return nc


_NC = None


def _get_nc():
    global _NC
    if _NC is None:
        _NC = build()
    return _NC


def _to_pmajor(block: np.ndarray) -> np.ndarray:
    # (512, 512) row-major -> (128, 4, 512): row r = t*P + p lands at
    # [p, t, :], so every SBUF partition's bytes are contiguous in DRAM.
    return block.reshape(T, P, B).transpose(1, 0, 2)


def make_in_maps(sims: np.ndarray) -> list[dict[str, np.ndarray]]:
    in_maps = []
    for c in range(NCORES):
        ab_stack = np.empty((NSLOTS, P, 2, T, B), np.float16)
        for k, (i, j) in enumerate(CORE_PAIRS[c]):
            if i != j:
                a = sims[i * B:(i + 1) * B, j * B:(j + 1) * B]
                bt = sims[j * B:(j + 1) * B, i * B:(i + 1) * B].T
            else:
                a = sims[i * B:(i + 1) * B, i * B:(i + 1) * B].copy()
                np.fill_diagonal(a, MASK)
                bt = a.T
            ab_stack[k, :, 0] = _to_pmajor(a).astype(np.float16)
            ab_stack[k, :, 1] = _to_pmajor(
                np.ascontiguousarray(bt)).astype(np.float16)
        in_maps.append({"ab": ab_stack})
    return in_maps


def assemble(results: list[dict[str, np.ndarray]]) -> np.ndarray:
    out = np.empty((N, N), np.float32)
    for c in range(NCORES):
        o_pm = results[c]["o"]  # (NSLOTS, P, T, B) fp16, partition-major
        o_stack = o_pm.astype(np.float32).transpose(0, 2, 1, 3).reshape(
            NSLOTS, B, B)
        for k, (i, j) in enumerate(CORE_PAIRS[c]):
            out[i * B:(i + 1) * B, j * B:(j + 1) * B] = o_stack[k]
            if i != j:
                out[j * B:(j + 1) * B, i * B:(i + 1) * B] = o_stack[k].T
    return out


def run_on_hw(sims: np.ndarray, **spmd_kwargs):
    """Run the kernel on the 8 NeuronCores. Returns (out, BassKernelResults).

    The device occasionally throws a transient NRT_EXEC_UNIT_UNRECOVERABLE
    and needs ~a minute to come back, so failed runs are retried."""
    import time

    nc = _get_nc()
    in_maps = make_in_maps(sims)
    last_exc = None
    for attempt in range(3):
        if attempt:
            time.sleep(75)
        try:
            res = run_bass_kernel_spmd(
                nc, in_maps, core_ids=list(range(NCORES)), **spmd_kwargs
            )
            return assemble(res.results), res
        except Exception as exc:  # noqa: BLE001 - device flake, retry
            last_exc = exc
    raise last_exc


def kernel(similarities: np.ndarray) -> np.ndarray:
    sims = np.ascontiguousarray(similarities, dtype=np.float32)
    assert sims.shape == (N, N)
    out, _ = run_on_hw(sims)
    return out


if __name__ == "__main__":
    rng = np.random.default_rng(0)
    sims = rng.standard_normal((N, N), dtype=np.float32)
    out = kernel(similarities=sims)
    print("out", out.shape, out.dtype, float(out.max()))
